# revision 5
# baseline (speedup 1.0000x reference)
"""Hamilton-Adams demosaic for Trainium2 — v2 (bf16, PE banded verticals, paired ops).

Input:  x [8, 4, 768, 768] f32  (quarter-res planes A=R, B=Gr, C=Gb, D=B)
Output: [8, 3, 1536, 1536] f32

All device compute in bf16 (host casts in/out).  Every gradient feeding a
directional-selection compare (vdif, c3, hdif, c2, dm, dn) runs on the Tensor
engine as a banded-stationary matmul accumulating BOTH the bf16 input and a
bf16 residual channel (xr = x - bf16(x)) in fp32 PSUM, so selection decisions
are effectively fp32-exact; only output values carry bf16 noise (l2 ~8e-3,
budget 2e-2).  Cross-partition (vertical) stencils and shifts are all PE
matmuls (W [128,128] per band, edge strips get modified bands + patch taps);
horizontal value ops run on DVE; |x| on Act; compares/copies on Pool.  The two
symmetric phases of each computation are fused into single wide ops via 3-dim
access patterns with a per-half column offset; PSUM results use 2-bank
single-half tiles on a 4-tag rotation for pipeline depth.

Tile layouts (halo col each side of every 770-wide plane region):
  P2  [128, 6*770] bf16: [au | A | B | C | D | dd]  (au=A row+1, dd=D row-1)
  P2r [128, 4*770+2] bf16: residual planes [A | B | C | D]
  G2  [128, 2*770] bf16: [g00 | g11]
  O   [128, 3*3072] bf16: interleaved full-res output rows (R | G | B)
"""
import sys
sys.path.insert(0, '/opt/trn_rl_repo')

from contextlib import ExitStack

import numpy as np
import ml_dtypes

import concourse.bass as bass
import concourse.bacc as bacc
import concourse.tile as tile
from concourse import mybir
from concourse.bass_utils import run_bass_kernel_spmd

F = mybir.dt.float32
BF = mybir.dt.bfloat16
U8 = mybir.dt.uint8
AL = mybir.AluOpType
AF = mybir.ActivationFunctionType
BFNP = ml_dtypes.bfloat16

H = 768
PW = 770
NCORES = 8
STRIPS = [0, 124, 248, 372, 496, 620, 644]

# P2 region base cols
AU, A_, B_, C_, D_, DD = 0, 770, 1540, 2310, 3080, 3850
# P2r (bf16 residual planes) region base cols — offset by 1 so taps with
# column delta -1/+1 on the first/last region stay inside the tile
RA, RB, RC, RD = 1, 771, 1541, 2311
# G2 region base cols
G00, G11 = 0, 770
# O color base cols
CR, CG, CB = 0, 3072, 6144

# ---------------- host-side band matrices ----------------
_mats: list[np.ndarray] = []
_mat_idx: dict = {}


def _W(taps=None, colfix=None):
    taps = taps or {}
    colfix = colfix or {}
    key = (tuple(sorted(taps.items())),
           tuple(sorted((m, tuple(sorted(f.items()))) for m, f in colfix.items())))
    if key in _mat_idx:
        return _mat_idx[key]
    M = np.zeros((128, 128), np.float32)
    for m in range(128):
        if m in colfix:
            for k, w in colfix[m].items():
                M[k, m] = w
        else:
            for d, w in taps.items():
                k = m + d
                if 0 <= k < 128:
                    M[k, m] = w
    idx = len(_mats)
    _mats.append(M)
    _mat_idx[key] = idx
    return idx


def _tap_table(kind):
    """kind in 'f' (first strip), 'm' (mid), 'l' (last).
    Returns result -> (half0 taps, half1 taps); tap = (Widx, src_region).
    src_region is a P2 or G2 base col; G-sourced taps marked by ('G', base)."""
    f, l = kind == 'f', kind == 'l'

    def FF(fix):
        return {0: {}, 1: {}, **fix}

    def LL(fix):
        return {**fix, 126: {}, 127: {}}

    def BW(taps, ffix=None, lfix=None):
        cf = None
        if f and ffix is not None:
            cf = FF(ffix)
        if l and lfix is not None:
            cf = LL(lfix)
        return _W(taps, cf)

    P22 = _W({}, {2: {2: 1.0}})
    P125 = _W({}, {125: {125: 1.0}})
    P125n = _W({}, {125: {125: -1.0}})
    P = lambda b: ('P', b)
    G = lambda b: ('G', b)
    R = lambda b: ('R', b)
    RES = {A_: RA, B_: RB, C_: RC, D_: RD}

    def with_res(taps):
        # duplicate every P-plane tap onto the residual tile (same W, same delta)
        out = list(taps)
        for wi, (sk, sb), dlt in taps:
            if sk == 'P':
                out.append((wi, R(RES[sb]), dlt))
        return out

    I1 = _W({0: 1.0})
    NegI = _W({0: -1.0})
    Neg2I = _W({0: -2.0})
    D1 = _W({1: 1.0}, LL({125: {}}) if l else None)   # down-shift; zero bottom edge
    U1 = _W({-1: 1.0}, FF({2: {}}) if f else None)    # up-shift; zero top edge
    NegU1 = _W({-1: -1.0}, FF({2: {}}) if f else None)

    T = {}
    # VD: vdif00 = C[i-1]-C[i]  |  vdif11 = B[i]-B[i+1]   (exact: + residual taps)
    T['VD'] = (
        with_res([(BW({-1: 1, 0: -1}, ffix={2: {2: -1}}), P(C_), 0)] + ([(P22, P(A_), 0)] if f else [])),
        with_res([(BW({0: 1, 1: -1}, lfix={125: {125: 1}}), P(B_), 0)] + ([(P125n, P(D_), 0)] if l else [])),
    )
    # C3: A[i-1]-2A[i]+A[i+1]  |  D[i-1]-2D[i]+D[i+1]
    T['C3'] = (
        with_res([(BW({-1: 1, 0: -2, 1: 1}, ffix={2: {2: -1, 3: 1}}, lfix={125: {124: 1, 125: -2}}), P(A_), 0)]
                 + ([(P125, P(C_), 0)] if l else [])),
        with_res([(BW({-1: 1, 0: -2, 1: 1}, ffix={2: {2: -2, 3: 1}}, lfix={125: {124: 1, 125: -1}}), P(D_), 0)]
                 + ([(P22, P(B_), 0)] if f else [])),
    )
    # HD: hdif00 = B[j-1]-B[j]  |  hdif11 = C[j]-C[j+1]   (horizontal, via col deltas)
    T['HD'] = (
        with_res([(I1, P(B_), -1), (NegI, P(B_), 0)]),
        with_res([(I1, P(C_), 0), (NegI, P(C_), 1)]),
    )
    # C2: A[j-1]-2A[j]+A[j+1]  |  D[j-1]-2D[j]+D[j+1]  (edge cols fixed on psum)
    T['C2'] = (
        with_res([(I1, P(A_), -1), (Neg2I, P(A_), 0), (I1, P(A_), 1)]),
        with_res([(I1, P(D_), -1), (Neg2I, P(D_), 0), (I1, P(D_), 1)]),
    )
    # RV: rawv = 0.5*vsum - 0.25*c3 (the vertical green candidate, final scale)
    T['RV'] = (
        [(BW({-1: .5, 0: .5}, ffix={2: {2: .5}}, lfix={125: {124: .5, 125: .25}}), P(C_), 0),
         (BW({-1: -.25, 0: .5, 1: -.25}, ffix={2: {2: .75, 3: -.25}},
             lfix={125: {124: -.25, 125: .5}}), P(A_), 0)],
        [(BW({0: .5, 1: .5}, ffix={2: {2: .25, 3: .5}}, lfix={125: {125: .5}}), P(B_), 0),
         (BW({-1: -.25, 0: .5, 1: -.25}, ffix={2: {2: .5, 3: -.25}},
             lfix={125: {124: -.25, 125: .75}}), P(D_), 0)],
    )
    # U = t1 - 0.5*t2 for the vertical hv pair (R10 | B01):
    #   R10: (A[i]+A[i+1]) - 0.5*(g00[i]+g00[i+1])   (bottom edge: A only / +C patch)
    #   B01: (D[i-1]+D[i]) - 0.5*(g11[i-1]+g11[i])   (top edge: D only / +B patch)
    P125h = _W({}, {125: {125: -0.5}})
    P22h = _W({}, {2: {2: -0.5}})
    T['U'] = (
        [(BW({0: 1, 1: 1}, lfix={125: {125: 1}}), P(A_), 0),
         (BW({0: -.5, 1: -.5}, lfix={125: {125: -.5}}), G(G00), 0)]
        + ([(P125h, P(C_), 0)] if l else []),
        [(BW({-1: 1, 0: 1}, ffix={2: {2: 1}}), P(D_), 0),
         (BW({-1: -.5, 0: -.5}, ffix={2: {2: -.5}}), G(G11), 0)]
        + ([(P22h, P(B_), 0)] if f else []),
    )
    # S: au = A[i+1] (last: 0)  |  dd = D[i-1] (first: 0)
    T['S'] = (
        [(D1, P(A_), 0)],
        [(U1, P(D_), 0)],
    )
    # DM: |R11| au[j+1]-A[j] = A[i+1,j+1]-A[i,j]  |  |B00| D[i,j]-D[i-1,j-1]
    T['DM'] = (
        with_res([(D1, P(A_), 1), (NegI, P(A_), 0)]),
        with_res([(I1, P(D_), 0), (NegU1, P(D_), -1)]),
    )
    # DN: R11: A[i+1,j]-A[i,j+1]  |  B00: D[i,j-1]-D[i-1,j]
    T['DN'] = (
        with_res([(D1, P(A_), 0), (NegI, P(A_), 1)]),
        with_res([(I1, P(D_), -1), (NegU1, P(D_), 0)]),
    )
    # CG2 = sp - 2*gnear;  sp_R11 = g00[i,j]+g00[i+1,j+1], sp_B00 = g11[i-1,j-1]+g11[i,j]
    T['CG2'] = (
        [(I1, G(G00), 0), (D1, G(G00), 1)] + ([(P125, P(C_), 1)] if l else [])
        + [(Neg2I, G(G11), 0)],
        [(U1, G(G11), -1)] + ([(P22, P(B_), -1)] if f else [])
        + [(I1, G(G11), 0), (Neg2I, G(G00), 0)],
    )
    # CG3 = sn - 2*gnear;  sn_R11 = g00[i,j+1]+g00[i+1,j], sn_B00 = g11[i-1,j]+g11[i,j-1]
    T['CG3'] = (
        [(I1, G(G00), 1), (D1, G(G00), 0)] + ([(P125, P(C_), 0)] if l else [])
        + [(Neg2I, G(G11), 0)],
        [(U1, G(G11), 0)] + ([(P22, P(B_), 0)] if f else [])
        + [(I1, G(G11), -1), (Neg2I, G(G00), 0)],
    )
    return T


_TAPS = {k: _tap_table(k) for k in ('f', 'm', 'l')}
NW = len(_mats)
WBANDS = np.concatenate([m.astype(BFNP) for m in _mats], axis=1)  # [128, NW*128]


def pv(t, o0, o1, w=768, step=1):
    """Pair view: [128][2 halves at col offsets o0/o1][w cols at `step`]."""
    b = t[:]
    assert o1 > o0, (o0, o1)
    return bass.AP(tensor=b.tensor, offset=b.offset + o0,
                   ap=[[b.ap[0][0], 128], [o1 - o0, 2], [step, w]])


def build_nc():
    nc = bacc.Bacc("TRN2", target_bir_lowering=False, debug=False, num_devices=NCORES)
    x_in = nc.declare_dram_parameter("x", [4, H, H], BF, isOutput=False)
    xr_in = nc.declare_dram_parameter("xr", [4, H, H], BF, isOutput=False)
    wb_in = nc.declare_dram_parameter("wb", [128, NW * 128], BF, isOutput=False)
    out = nc.declare_dram_parameter("out", [3, 2 * H, 2 * H], BF, isOutput=True)
    out_v = out[:].rearrange("c (r two) w -> c r (two w)", two=2)
    x_r = x_in[:].rearrange("c r w -> r c w")
    xr_r = xr_in[:].rearrange("c r w -> r c w")

    with tile.TileContext(nc) as tc, ExitStack() as ctx:
        wpool = ctx.enter_context(tc.tile_pool(name="wpool", bufs=1))
        pl = ctx.enter_context(tc.tile_pool(name="planes", bufs=2))
        gp = ctx.enter_context(tc.tile_pool(name="greens", bufs=2))
        tp = ctx.enter_context(tc.tile_pool(name="tmps", bufs=16))
        qp_ = ctx.enter_context(tc.tile_pool(name="qtmps", bufs=6))
        mp = ctx.enter_context(tc.tile_pool(name="msks", bufs=2))
        op = ctx.enter_context(tc.tile_pool(name="outs", bufs=2))
        pp = ctx.enter_context(tc.tile_pool(name="psums", bufs=1, space="PSUM"))

        WS = wpool.tile([128, NW * 128], BF, tag="ws", name="ws")
        nc.sync.dma_start(WS[:], wb_in[:])

        for si, r0 in enumerate(STRIPS):
            kind = 'f' if si == 0 else ('l' if si == len(STRIPS) - 1 else 'm')
            taps = _TAPS[kind]
            base = r0 - 2

            psum_ctr = [0]
            P2 = pl.tile([128, 6 * PW], BF, tag="P2", name=f"P2_{si}")
            P2r = pl.tile([128, 4 * PW + 2], BF, tag="P2r", name=f"P2r_{si}")
            G2 = gp.tile([128, 2 * PW], BF, tag="G2", name=f"G2_{si}")
            O = op.tile([128, 3 * 3072], BF, tag="O", name=f"O_{si}")

            def tmp(tag):
                return tp.tile([128, 1536], BF, tag="t", name=f"{tag}_{si}")

            def qf(tag):
                return qp_.tile([128, 1536], F, tag="q", name=f"{tag}_{si}")

            def msk(tag):
                return mp.tile([128, 1536], U8, tag=tag, name=f"{tag}_{si}")

            def psum(tag):
                i = psum_ctr[0] % 4
                psum_ctr[0] += 1
                return pp.tile([128, 768], F, tag=f"u{i}", name=f"{tag}_{si}")

            def emit_half(ps, tl):
                for (c0, c1) in ((0, 512), (512, 768)):
                    n = len(tl)
                    for i, (wi, (sk, sb), dlt) in enumerate(tl):
                        src = {'P': P2, 'G': G2, 'R': P2r}[sk]
                        nc.tensor.matmul(
                            ps[:, c0:c1],
                            WS[:, wi * 128:(wi + 1) * 128],
                            src[:, sb + 1 + dlt + c0:sb + 1 + dlt + c1],
                            start=(i == 0), stop=(i == n - 1))

            # ---- input DMA (planes A..D at P2 cols 770..3849) ----
            # compute-engine partition starts must be 32-aligned: memset a whole
            # 32-lane block first, then let the DMA overwrite the loaded lanes.
            if kind == 'f':
                nc.gpsimd.memset(P2[0:32, :], 0.0)
                nc.gpsimd.memset(P2r[0:32, :], 0.0)
            if kind == 'l':
                nc.gpsimd.memset(P2[96:128, :], 0.0)
                nc.gpsimd.memset(P2r[96:128, :], 0.0)
            clo, chi = max(base, 0), min(base + 128, H)
            Pr = P2[:].rearrange("p (six w) -> p six w", six=6)
            nc.sync.dma_start(Pr[clo - base: chi - base, 1:5, 1:1 + H], x_r[clo:chi, :, :])
            _b = P2r[clo - base: chi - base, :]
            Prr = bass.AP(tensor=_b.tensor, offset=_b.offset + RA + 1,
                          ap=[_b.ap[0], [PW, 4], [1, H]])
            nc.sync.dma_start(Prr, xr_r[clo:chi, :, :])

            # ---- plane halo cols ----
            # A.r and D.l are ZERO (masked-channel replication) — the two wide
            # ops that instead need the mosaic replication (c2 pair) get tiny
            # single-column fixups below.
            cc = nc.vector.tensor_copy
            cc(P2[:, A_:A_ + 1], P2[:, A_ + 1:A_ + 2])          # A.l = A[0]
            cc(P2[:, B_:B_ + 1], P2[:, A_ + 1:A_ + 2])          # B.l = A[0]
            nc.vector.memset(P2[:, D_:D_ + 1], 0.0)             # D.l = 0
            nc.vector.memset(P2[:, A_ + 769:A_ + 770], 0.0)     # A.r = 0
            cc(P2[:, C_ + 769:C_ + 770], P2[:, D_ + 768:D_ + 769])  # C.r = D[767]
            cc(P2[:, D_ + 769:D_ + 770], P2[:, D_ + 768:D_ + 769])  # D.r = D[767]
            cc(P2r[:, RA:RA + 1], P2r[:, RA + 1:RA + 2])
            cc(P2r[:, RB:RB + 1], P2r[:, RA + 1:RA + 2])
            nc.vector.memset(P2r[:, RD:RD + 1], 0.0)
            nc.vector.memset(P2r[:, RA + 769:RA + 770], 0.0)
            cc(P2r[:, RC + 769:RC + 770], P2r[:, RD + 768:RD + 769])
            cc(P2r[:, RD + 769:RD + 770], P2r[:, RD + 768:RD + 769])

            # ---- green: PE verticals (consumers emitted in psum-alloc order
            # so the in-order ACT queue can never cycle with PE buffer reuse) ----
            tV = taps['VD']; tC = taps['C3']; tH = taps['HD']; tW = taps['C2']
            psVD0 = psum("vd0"); emit_half(psVD0, tV[0])
            q5 = qf("q5"); nc.scalar.activation(q5[:, 0:768], psVD0[:], AF.Abs)
            psVD1 = psum("vd1"); emit_half(psVD1, tV[1])
            nc.scalar.activation(q5[:, 768:1536], psVD1[:], AF.Abs)
            psC30 = psum("c30"); emit_half(psC30, tC[0])
            q3 = qf("q3"); nc.scalar.activation(q3[:, 0:768], psC30[:], AF.Abs)
            psC31 = psum("c31"); emit_half(psC31, tC[1])
            nc.scalar.activation(q3[:, 768:1536], psC31[:], AF.Abs)

            # ---- green: horizontals ----
            hs = tmp("hs"); nc.vector.tensor_tensor(hs[:], pv(P2, B_, C_ + 1), pv(P2, B_ + 1, C_ + 2), AL.add)
            psHD0 = psum("hd0"); emit_half(psHD0, tH[0])
            q4 = qf("q4"); nc.scalar.activation(q4[:, 0:768], psHD0[:], AF.Abs)
            psHD1 = psum("hd1"); emit_half(psHD1, tH[1])
            nc.scalar.activation(q4[:, 768:1536], psHD1[:], AF.Abs)
            psC20 = psum("c20"); emit_half(psC20, tW[0])
            # c2 wants mosaic replication at the zeroed A.r/D.l halos (main+residual):
            nc.vector.tensor_tensor(psC20[:, 767:768], psC20[:, 767:768], P2[:, B_ + 768:B_ + 769], AL.add)
            nc.vector.tensor_tensor(psC20[:, 767:768], psC20[:, 767:768], P2r[:, RB + 768:RB + 769], AL.add)
            b2 = tmp("b2"); nc.vector.scalar_tensor_tensor(b2[:, 0:768], psC20[:], -0.5, hs[:, 0:768], AL.mult, AL.add)
            q2 = qf("q2"); nc.scalar.activation(q2[:, 0:768], psC20[:], AF.Abs)
            psC21 = psum("c21"); emit_half(psC21, tW[1])
            nc.vector.tensor_tensor(psC21[:, 0:1], psC21[:, 0:1], P2[:, C_ + 1:C_ + 2], AL.add)
            nc.vector.tensor_tensor(psC21[:, 0:1], psC21[:, 0:1], P2r[:, RC + 1:RC + 2], AL.add)
            nc.vector.scalar_tensor_tensor(b2[:, 768:1536], psC21[:], -0.5, hs[:, 768:1536], AL.mult, AL.add)
            nc.scalar.activation(q2[:, 768:1536], psC21[:], AF.Abs)

            clh = qf("clh"); nc.gpsimd.tensor_tensor(clh[:], q4[:], q2[:], AL.add)
            clv = qf("clv"); nc.gpsimd.tensor_tensor(clv[:], q5[:], q3[:], AL.add)
            dt = qf("dt"); nc.gpsimd.tensor_tensor(dt[:], clh[:], clv[:], AL.subtract)
            mk = msk("mk"); nc.vector.tensor_scalar(mk[:], dt[:], 0.0, None, AL.is_gt)

            tR = taps['RV']
            gint = pv(G2, G00 + 1, G11 + 1)
            nc.vector.tensor_scalar(gint, b2[:], 0.5, None, AL.mult)
            psRV0 = psum("rv0"); emit_half(psRV0, tR[0])
            nc.vector.copy_predicated(G2[:, G00 + 1:G00 + 769], mk[:, 0:768], psRV0[:])
            psRV1 = psum("rv1"); emit_half(psRV1, tR[1])
            nc.vector.copy_predicated(G2[:, G11 + 1:G11 + 769], mk[:, 768:1536], psRV1[:])

            # green halo cols (before PE reads G2)
            cc(G2[:, G00 + 769:G00 + 770], P2[:, B_ + 768:B_ + 769])  # g00.r = B[767]
            cc(G2[:, G11:G11 + 1], P2[:, C_ + 1:C_ + 2])              # g11.l = C[0]

            # green outputs: raw (0,1),(1,0) and computed (0,0),(1,1)
            nc.gpsimd.tensor_copy(pv(O, CG + 1, CG + 1536, step=2), pv(P2, B_ + 1, C_ + 1))
            nc.gpsimd.tensor_copy(pv(O, CG, CG + 1537, step=2), gint)

            # ---- hv fields ----
            tU = taps['U']
            psU0 = psum("u0"); emit_half(psU0, tU[0])
            psU1 = psum("u1"); emit_half(psU1, tU[1])

            t1h = tmp("t1h"); nc.vector.tensor_tensor(t1h[:], pv(P2, A_ + 1, D_), pv(P2, A_ + 2, D_ + 1), AL.add)
            t2h = tmp("t2h"); nc.vector.tensor_tensor(t2h[:], pv(G2, G00 + 1, G11), pv(G2, G00 + 2, G11 + 1), AL.add)
            uh = tmp("uh"); nc.vector.scalar_tensor_tensor(uh[:], t2h[:], -0.5, t1h[:], AL.mult, AL.add)

            hbc = tmp("hbc"); nc.vector.tensor_scalar(hbc[:], pv(P2, B_ + 1, C_ + 1), 0.5, None, AL.mult)
            # [C|B] pair is order-reversed in P2; compute halves separately into one tmp
            hcb = tmp("hcb")
            nc.vector.tensor_scalar(hcb[:, 0:768], P2[:, C_ + 1:C_ + 769], 0.5, None, AL.mult)
            nc.vector.tensor_scalar(hcb[:, 768:1536], P2[:, B_ + 1:B_ + 769], 0.5, None, AL.mult)

            nc.vector.scalar_tensor_tensor(pv(O, CR + 1, CB + 1536, step=2), uh[:], 0.5, hbc[:], AL.mult, AL.add)
            _d = pv(O, CR + 1536, CB + 1, step=2)
            _d0 = bass.AP(tensor=_d.tensor, offset=_d.offset, ap=[_d.ap[0], [2, 768]])
            _d1 = bass.AP(tensor=_d.tensor, offset=_d.offset + (CB + 1 - (CR + 1536)), ap=[_d.ap[0], [2, 768]])
            nc.vector.scalar_tensor_tensor(_d0, psU0[:], 0.5, hcb[:, 0:768], AL.mult, AL.add)
            nc.vector.scalar_tensor_tensor(_d1, psU1[:], 0.5, hcb[:, 768:1536], AL.mult, AL.add)

            # ---- chan (R11 | B00) ----
            tS = taps['S']
            psS0 = psum("s0"); emit_half(psS0, tS[0])
            nc.scalar.copy(P2[:, AU + 1:AU + 769], psS0[:])
            nc.vector.memset(P2[:, AU + 769:AU + 770], 0.0)
            psS1 = psum("s1"); emit_half(psS1, tS[1])
            nc.scalar.copy(P2[:, DD + 1:DD + 769], psS1[:])
            nc.vector.memset(P2[:, DD:DD + 1], 0.0)

            rp = tmp("rp"); nc.vector.tensor_tensor(rp[:], pv(P2, A_ + 1, DD), pv(P2, AU + 2, D_ + 1), AL.add)
            rn = tmp("rn"); nc.vector.tensor_tensor(rn[:], pv(P2, A_ + 2, DD + 1), pv(P2, AU + 1, D_), AL.add)

            tDM = taps['DM']; tDN = taps['DN']
            psDM0 = psum("dm0"); emit_half(psDM0, tDM[0])
            qdm = qf("qdm"); nc.scalar.activation(qdm[:, 0:768], psDM0[:], AF.Abs)
            psDM1 = psum("dm1"); emit_half(psDM1, tDM[1])
            nc.scalar.activation(qdm[:, 768:1536], psDM1[:], AF.Abs)
            psDN0 = psum("dn0"); emit_half(psDN0, tDN[0])
            qdn = qf("qdn"); nc.scalar.activation(qdn[:, 0:768], psDN0[:], AF.Abs)
            psDN1 = psum("dn1"); emit_half(psDN1, tDN[1])
            nc.scalar.activation(qdn[:, 768:1536], psDN1[:], AF.Abs)

            tG2_ = taps['CG2']; tG3_ = taps['CG3']
            psCG20 = psum("g20"); emit_half(psCG20, tG2_[0])
            qp = qf("qp"); nc.scalar.activation(qp[:, 0:768], psCG20[:], AF.Abs)
            cp2 = tmp("cp2"); nc.vector.scalar_tensor_tensor(cp2[:, 0:768], psCG20[:], -0.5, rp[:, 0:768], AL.mult, AL.add)
            psCG21 = psum("g21"); emit_half(psCG21, tG2_[1])
            nc.scalar.activation(qp[:, 768:1536], psCG21[:], AF.Abs)
            nc.vector.scalar_tensor_tensor(cp2[:, 768:1536], psCG21[:], -0.5, rp[:, 768:1536], AL.mult, AL.add)
            psCG30 = psum("g30"); emit_half(psCG30, tG3_[0])
            qn = qf("qn"); nc.scalar.activation(qn[:, 0:768], psCG30[:], AF.Abs)
            cn2 = tmp("cn2"); nc.vector.scalar_tensor_tensor(cn2[:, 0:768], psCG30[:], -0.5, rn[:, 0:768], AL.mult, AL.add)
            psCG31 = psum("g31"); emit_half(psCG31, tG3_[1])
            nc.scalar.activation(qn[:, 768:1536], psCG31[:], AF.Abs)
            nc.vector.scalar_tensor_tensor(cn2[:, 768:1536], psCG31[:], -0.5, rn[:, 768:1536], AL.mult, AL.add)
            clp = qf("clp"); nc.gpsimd.tensor_tensor(clp[:], qdm[:], qp[:], AL.add)
            cln = qf("cln"); nc.gpsimd.tensor_tensor(cln[:], qdn[:], qn[:], AL.add)
            dr = qf("dr"); nc.gpsimd.tensor_tensor(dr[:], clp[:], cln[:], AL.subtract)
            mr = msk("mr"); nc.vector.tensor_scalar(mr[:], dr[:], 0.0, None, AL.is_gt)
            cnh = tmp("cnh"); nc.vector.tensor_scalar(cnh[:], cn2[:], 0.5, None, AL.mult)
            odst = pv(O, CR + 1537, CB, step=2)
            nc.scalar.mul(odst, cp2[:], 0.5)
            nc.vector.copy_predicated(odst, mr[:], cnh[:])

            # ---- raw red/blue phases ----
            nc.gpsimd.tensor_copy(pv(O, CR, CB + 1537, step=2), pv(P2, A_ + 1, D_ + 1))

            # ---- output DMA (3 colors on 3 queues) ----
            if kind == 'l':
                p0, pn, row0 = 102, 24, 744
            else:
                p0, pn, row0 = 2, 124, r0
            nc.sync.dma_start(out_v[0, row0:row0 + pn, :], O[p0:p0 + pn, CR:CR + 3072])
            nc.scalar.dma_start(out_v[1, row0:row0 + pn, :], O[p0:p0 + pn, CG:CG + 3072])
            nc.gpsimd.dma_start(out_v[2, row0:row0 + pn, :], O[p0:p0 + pn, CB:CB + 3072])

    nc.compile()
    return nc


_NC_CACHE = None


def kernel(x: np.ndarray) -> np.ndarray:
    global _NC_CACHE
    if _NC_CACHE is None:
        _NC_CACHE = build_nc()
    xb = np.ascontiguousarray(x.astype(BFNP))
    xr = np.ascontiguousarray((x.astype(np.float32) - xb.astype(np.float32)).astype(BFNP))
    wb = np.ascontiguousarray(WBANDS)
    in_maps = [{"x": xb[i], "xr": xr[i], "wb": wb} for i in range(NCORES)]
    res = run_bass_kernel_spmd(_NC_CACHE, in_maps, list(range(NCORES)))
    return np.stack([res.results[i]["out"].astype(np.float32) for i in range(NCORES)], axis=0)


# revision 6
# speedup vs baseline: 1.0147x; 1.0147x over previous
"""Hamilton-Adams demosaic for Trainium2 — v2 (bf16, PE banded verticals, paired ops).

Input:  x [8, 4, 768, 768] f32  (quarter-res planes A=R, B=Gr, C=Gb, D=B)
Output: [8, 3, 1536, 1536] f32

All device compute in bf16 (host casts in/out).  Every gradient feeding a
directional-selection compare (vdif, c3, hdif, c2, dm, dn) runs on the Tensor
engine as a banded-stationary matmul accumulating BOTH the bf16 input and a
bf16 residual channel (xr = x - bf16(x)) in fp32 PSUM, so selection decisions
are effectively fp32-exact; only output values carry bf16 noise (l2 ~8e-3,
budget 2e-2).  Cross-partition (vertical) stencils and shifts are all PE
matmuls (W [128,128] per band, edge strips get modified bands + patch taps);
horizontal value ops run on DVE; |x| on Act; compares/copies on Pool.  The two
symmetric phases of each computation are fused into single wide ops via 3-dim
access patterns with a per-half column offset; PSUM results use 2-bank
single-half tiles on a 4-tag rotation for pipeline depth.

Tile layouts (halo col each side of every 770-wide plane region):
  P2  [128, 6*770] bf16: [au | A | B | C | D | dd]  (au=A row+1, dd=D row-1)
  P2r [128, 4*770+2] bf16: residual planes [A | B | C | D]
  G2  [128, 2*770] bf16: [g00 | g11]
  O   [128, 3*3072] bf16: interleaved full-res output rows (R | G | B)
"""
import sys
sys.path.insert(0, '/opt/trn_rl_repo')

from contextlib import ExitStack

import numpy as np
import ml_dtypes

import concourse.bass as bass
import concourse.bacc as bacc
import concourse.tile as tile
from concourse import mybir
from concourse.bass_utils import run_bass_kernel_spmd

F = mybir.dt.float32
BF = mybir.dt.bfloat16
U8 = mybir.dt.uint8
AL = mybir.AluOpType
AF = mybir.ActivationFunctionType
BFNP = ml_dtypes.bfloat16

H = 768
PW = 770
NCORES = 8
STRIPS = [0, 124, 248, 372, 496, 620, 644]

# P2 region base cols
AU, A_, B_, C_, D_, DD = 0, 770, 1540, 2310, 3080, 3850
# P2r (bf16 residual planes) region base cols — offset by 1 so taps with
# column delta -1/+1 on the first/last region stay inside the tile
RA, RB, RC, RD = 1, 771, 1541, 2311
# G2 region base cols
G00, G11 = 0, 770
# O color base cols
CR, CG, CB = 0, 3072, 6144

# ---------------- host-side band matrices ----------------
_mats: list[np.ndarray] = []
_mat_idx: dict = {}


def _W(taps=None, colfix=None):
    taps = taps or {}
    colfix = colfix or {}
    key = (tuple(sorted(taps.items())),
           tuple(sorted((m, tuple(sorted(f.items()))) for m, f in colfix.items())))
    if key in _mat_idx:
        return _mat_idx[key]
    M = np.zeros((128, 128), np.float32)
    for m in range(128):
        if m in colfix:
            for k, w in colfix[m].items():
                M[k, m] = w
        else:
            for d, w in taps.items():
                k = m + d
                if 0 <= k < 128:
                    M[k, m] = w
    idx = len(_mats)
    _mats.append(M)
    _mat_idx[key] = idx
    return idx


def _tap_table(kind):
    """kind in 'f' (first strip), 'm' (mid), 'l' (last).
    Returns result -> (half0 taps, half1 taps); tap = (Widx, src_region).
    src_region is a P2 or G2 base col; G-sourced taps marked by ('G', base)."""
    f, l = kind == 'f', kind == 'l'

    def FF(fix):
        return {0: {}, 1: {}, **fix}

    def LL(fix):
        return {**fix, 126: {}, 127: {}}

    def BW(taps, ffix=None, lfix=None):
        cf = None
        if f and ffix is not None:
            cf = FF(ffix)
        if l and lfix is not None:
            cf = LL(lfix)
        return _W(taps, cf)

    P22 = _W({}, {2: {2: 1.0}})
    P125 = _W({}, {125: {125: 1.0}})
    P125n = _W({}, {125: {125: -1.0}})
    P = lambda b: ('P', b)
    G = lambda b: ('G', b)
    R = lambda b: ('R', b)
    RES = {A_: RA, B_: RB, C_: RC, D_: RD}

    def with_res(taps):
        # duplicate every P-plane tap onto the residual tile (same W, same delta)
        out = list(taps)
        for wi, (sk, sb), dlt in taps:
            if sk == 'P':
                out.append((wi, R(RES[sb]), dlt))
        return out

    I1 = _W({0: 1.0})
    NegI = _W({0: -1.0})
    Neg2I = _W({0: -2.0})
    D1 = _W({1: 1.0}, LL({125: {}}) if l else None)   # down-shift; zero bottom edge
    U1 = _W({-1: 1.0}, FF({2: {}}) if f else None)    # up-shift; zero top edge
    NegU1 = _W({-1: -1.0}, FF({2: {}}) if f else None)

    T = {}
    # VD: vdif00 = C[i-1]-C[i]  |  vdif11 = B[i]-B[i+1]   (exact: + residual taps)
    T['VD'] = (
        with_res([(BW({-1: 1, 0: -1}, ffix={2: {2: -1}}), P(C_), 0)] + ([(P22, P(A_), 0)] if f else [])),
        with_res([(BW({0: 1, 1: -1}, lfix={125: {125: 1}}), P(B_), 0)] + ([(P125n, P(D_), 0)] if l else [])),
    )
    # C3: A[i-1]-2A[i]+A[i+1]  |  D[i-1]-2D[i]+D[i+1]
    T['C3'] = (
        with_res([(BW({-1: 1, 0: -2, 1: 1}, ffix={2: {2: -1, 3: 1}}, lfix={125: {124: 1, 125: -2}}), P(A_), 0)]
                 + ([(P125, P(C_), 0)] if l else [])),
        with_res([(BW({-1: 1, 0: -2, 1: 1}, ffix={2: {2: -2, 3: 1}}, lfix={125: {124: 1, 125: -1}}), P(D_), 0)]
                 + ([(P22, P(B_), 0)] if f else [])),
    )
    # HD: hdif00 = B[j-1]-B[j]  |  hdif11 = C[j]-C[j+1]   (horizontal, via col deltas)
    T['HD'] = (
        with_res([(I1, P(B_), -1), (NegI, P(B_), 0)]),
        with_res([(I1, P(C_), 0), (NegI, P(C_), 1)]),
    )
    # C2: A[j-1]-2A[j]+A[j+1]  |  D[j-1]-2D[j]+D[j+1]  (edge cols fixed on psum)
    T['C2'] = (
        with_res([(I1, P(A_), -1), (Neg2I, P(A_), 0), (I1, P(A_), 1)]),
        with_res([(I1, P(D_), -1), (Neg2I, P(D_), 0), (I1, P(D_), 1)]),
    )
    # RV: rawv = 0.5*vsum - 0.25*c3 (the vertical green candidate, final scale)
    T['RV'] = (
        [(BW({-1: .5, 0: .5}, ffix={2: {2: .5}}, lfix={125: {124: .5, 125: .25}}), P(C_), 0),
         (BW({-1: -.25, 0: .5, 1: -.25}, ffix={2: {2: .75, 3: -.25}},
             lfix={125: {124: -.25, 125: .5}}), P(A_), 0)],
        [(BW({0: .5, 1: .5}, ffix={2: {2: .25, 3: .5}}, lfix={125: {125: .5}}), P(B_), 0),
         (BW({-1: -.25, 0: .5, 1: -.25}, ffix={2: {2: .5, 3: -.25}},
             lfix={125: {124: -.25, 125: .75}}), P(D_), 0)],
    )
    # U = t1 - 0.5*t2 for the vertical hv pair (R10 | B01):
    #   R10: (A[i]+A[i+1]) - 0.5*(g00[i]+g00[i+1])   (bottom edge: A only / +C patch)
    #   B01: (D[i-1]+D[i]) - 0.5*(g11[i-1]+g11[i])   (top edge: D only / +B patch)
    P125h = _W({}, {125: {125: -0.5}})
    P22h = _W({}, {2: {2: -0.5}})
    T['U'] = (
        [(BW({0: 1, 1: 1}, lfix={125: {125: 1}}), P(A_), 0),
         (BW({0: -.5, 1: -.5}, lfix={125: {125: -.5}}), G(G00), 0)]
        + ([(P125h, P(C_), 0)] if l else []),
        [(BW({-1: 1, 0: 1}, ffix={2: {2: 1}}), P(D_), 0),
         (BW({-1: -.5, 0: -.5}, ffix={2: {2: -.5}}), G(G11), 0)]
        + ([(P22h, P(B_), 0)] if f else []),
    )
    # S: au = A[i+1] (last: 0)  |  dd = D[i-1] (first: 0)
    T['S'] = (
        [(D1, P(A_), 0)],
        [(U1, P(D_), 0)],
    )
    # DM: |R11| au[j+1]-A[j] = A[i+1,j+1]-A[i,j]  |  |B00| D[i,j]-D[i-1,j-1]
    T['DM'] = (
        [(D1, P(A_), 1), (NegI, P(A_), 0)],
        [(I1, P(D_), 0), (NegU1, P(D_), -1)],
    )
    # DN: R11: A[i+1,j]-A[i,j+1]  |  B00: D[i,j-1]-D[i-1,j]
    T['DN'] = (
        [(D1, P(A_), 0), (NegI, P(A_), 1)],
        [(I1, P(D_), -1), (NegU1, P(D_), 0)],
    )
    # CG2 = sp - 2*gnear;  sp_R11 = g00[i,j]+g00[i+1,j+1], sp_B00 = g11[i-1,j-1]+g11[i,j]
    T['CG2'] = (
        [(I1, G(G00), 0), (D1, G(G00), 1)] + ([(P125, P(C_), 1)] if l else [])
        + [(Neg2I, G(G11), 0)],
        [(U1, G(G11), -1)] + ([(P22, P(B_), -1)] if f else [])
        + [(I1, G(G11), 0), (Neg2I, G(G00), 0)],
    )
    # CG3 = sn - 2*gnear;  sn_R11 = g00[i,j+1]+g00[i+1,j], sn_B00 = g11[i-1,j]+g11[i,j-1]
    T['CG3'] = (
        [(I1, G(G00), 1), (D1, G(G00), 0)] + ([(P125, P(C_), 0)] if l else [])
        + [(Neg2I, G(G11), 0)],
        [(U1, G(G11), 0)] + ([(P22, P(B_), 0)] if f else [])
        + [(I1, G(G11), -1), (Neg2I, G(G00), 0)],
    )
    return T


_TAPS = {k: _tap_table(k) for k in ('f', 'm', 'l')}
NW = len(_mats)
WBANDS = np.concatenate([m.astype(BFNP) for m in _mats], axis=1)  # [128, NW*128]


def pv(t, o0, o1, w=768, step=1):
    """Pair view: [128][2 halves at col offsets o0/o1][w cols at `step`]."""
    b = t[:]
    assert o1 > o0, (o0, o1)
    return bass.AP(tensor=b.tensor, offset=b.offset + o0,
                   ap=[[b.ap[0][0], 128], [o1 - o0, 2], [step, w]])


def build_nc():
    nc = bacc.Bacc("TRN2", target_bir_lowering=False, debug=False, num_devices=NCORES)
    x_in = nc.declare_dram_parameter("x", [4, H, H], BF, isOutput=False)
    xr_in = nc.declare_dram_parameter("xr", [4, H, H], BF, isOutput=False)
    wb_in = nc.declare_dram_parameter("wb", [128, NW * 128], BF, isOutput=False)
    out = nc.declare_dram_parameter("out", [3, 2 * H, 2 * H], BF, isOutput=True)
    out_v = out[:].rearrange("c (r two) w -> c r (two w)", two=2)
    x_r = x_in[:].rearrange("c r w -> r c w")
    xr_r = xr_in[:].rearrange("c r w -> r c w")

    with tile.TileContext(nc) as tc, ExitStack() as ctx:
        wpool = ctx.enter_context(tc.tile_pool(name="wpool", bufs=1))
        pl = ctx.enter_context(tc.tile_pool(name="planes", bufs=2))
        gp = ctx.enter_context(tc.tile_pool(name="greens", bufs=2))
        tp = ctx.enter_context(tc.tile_pool(name="tmps", bufs=16))
        qp_ = ctx.enter_context(tc.tile_pool(name="qtmps", bufs=6))
        mp = ctx.enter_context(tc.tile_pool(name="msks", bufs=2))
        op = ctx.enter_context(tc.tile_pool(name="outs", bufs=2))
        pp = ctx.enter_context(tc.tile_pool(name="psums", bufs=1, space="PSUM"))

        WS = wpool.tile([128, NW * 128], BF, tag="ws", name="ws")
        nc.sync.dma_start(WS[:], wb_in[:])

        for si, r0 in enumerate(STRIPS):
            kind = 'f' if si == 0 else ('l' if si == len(STRIPS) - 1 else 'm')
            taps = _TAPS[kind]
            base = r0 - 2

            psum_ctr = [0]
            P2 = pl.tile([128, 6 * PW], BF, tag="P2", name=f"P2_{si}")
            P2r = pl.tile([128, 4 * PW + 2], BF, tag="P2r", name=f"P2r_{si}")
            G2 = gp.tile([128, 2 * PW], BF, tag="G2", name=f"G2_{si}")
            O = op.tile([128, 3 * 3072], BF, tag="O", name=f"O_{si}")

            def tmp(tag):
                return tp.tile([128, 1536], BF, tag="t", name=f"{tag}_{si}")

            def qf(tag):
                return qp_.tile([128, 1536], F, tag="q", name=f"{tag}_{si}")

            def msk(tag):
                return mp.tile([128, 1536], U8, tag=tag, name=f"{tag}_{si}")

            def psum(tag):
                i = psum_ctr[0] % 4
                psum_ctr[0] += 1
                return pp.tile([128, 768], F, tag=f"u{i}", name=f"{tag}_{si}")

            def emit_half(ps, tl):
                for (c0, c1) in ((0, 512), (512, 768)):
                    n = len(tl)
                    for i, (wi, (sk, sb), dlt) in enumerate(tl):
                        src = {'P': P2, 'G': G2, 'R': P2r}[sk]
                        nc.tensor.matmul(
                            ps[:, c0:c1],
                            WS[:, wi * 128:(wi + 1) * 128],
                            src[:, sb + 1 + dlt + c0:sb + 1 + dlt + c1],
                            start=(i == 0), stop=(i == n - 1))

            # ---- input DMA (planes A..D at P2 cols 770..3849) ----
            # compute-engine partition starts must be 32-aligned: memset a whole
            # 32-lane block first, then let the DMA overwrite the loaded lanes.
            if kind == 'f':
                nc.gpsimd.memset(P2[0:32, :], 0.0)
                nc.gpsimd.memset(P2r[0:32, :], 0.0)
            if kind == 'l':
                nc.gpsimd.memset(P2[96:128, :], 0.0)
                nc.gpsimd.memset(P2r[96:128, :], 0.0)
            clo, chi = max(base, 0), min(base + 128, H)
            Pr = P2[:].rearrange("p (six w) -> p six w", six=6)
            nc.sync.dma_start(Pr[clo - base: chi - base, 1:5, 1:1 + H], x_r[clo:chi, :, :])
            _b = P2r[clo - base: chi - base, :]
            Prr = bass.AP(tensor=_b.tensor, offset=_b.offset + RA + 1,
                          ap=[_b.ap[0], [PW, 4], [1, H]])
            nc.sync.dma_start(Prr, xr_r[clo:chi, :, :])

            # ---- plane halo cols ----
            # A.r and D.l are ZERO (masked-channel replication) — the two wide
            # ops that instead need the mosaic replication (c2 pair) get tiny
            # single-column fixups below.
            cc = nc.vector.tensor_copy
            cc(P2[:, A_:A_ + 1], P2[:, A_ + 1:A_ + 2])          # A.l = A[0]
            cc(P2[:, B_:B_ + 1], P2[:, A_ + 1:A_ + 2])          # B.l = A[0]
            nc.vector.memset(P2[:, D_:D_ + 1], 0.0)             # D.l = 0
            nc.vector.memset(P2[:, A_ + 769:A_ + 770], 0.0)     # A.r = 0
            cc(P2[:, C_ + 769:C_ + 770], P2[:, D_ + 768:D_ + 769])  # C.r = D[767]
            cc(P2[:, D_ + 769:D_ + 770], P2[:, D_ + 768:D_ + 769])  # D.r = D[767]
            cc(P2r[:, RA:RA + 1], P2r[:, RA + 1:RA + 2])
            cc(P2r[:, RB:RB + 1], P2r[:, RA + 1:RA + 2])
            nc.vector.memset(P2r[:, RD:RD + 1], 0.0)
            nc.vector.memset(P2r[:, RA + 769:RA + 770], 0.0)
            cc(P2r[:, RC + 769:RC + 770], P2r[:, RD + 768:RD + 769])
            cc(P2r[:, RD + 769:RD + 770], P2r[:, RD + 768:RD + 769])

            # ---- green: PE verticals (consumers emitted in psum-alloc order
            # so the in-order ACT queue can never cycle with PE buffer reuse) ----
            tV = taps['VD']; tC = taps['C3']; tH = taps['HD']; tW = taps['C2']
            psVD0 = psum("vd0"); emit_half(psVD0, tV[0])
            q5 = qf("q5"); nc.scalar.activation(q5[:, 0:768], psVD0[:], AF.Abs)
            psVD1 = psum("vd1"); emit_half(psVD1, tV[1])
            nc.scalar.activation(q5[:, 768:1536], psVD1[:], AF.Abs)
            psC30 = psum("c30"); emit_half(psC30, tC[0])
            q3 = qf("q3"); nc.scalar.activation(q3[:, 0:768], psC30[:], AF.Abs)
            psC31 = psum("c31"); emit_half(psC31, tC[1])
            nc.scalar.activation(q3[:, 768:1536], psC31[:], AF.Abs)

            # ---- green: horizontals ----
            hs = tmp("hs"); nc.vector.tensor_tensor(hs[:], pv(P2, B_, C_ + 1), pv(P2, B_ + 1, C_ + 2), AL.add)
            psHD0 = psum("hd0"); emit_half(psHD0, tH[0])
            q4 = qf("q4"); nc.scalar.activation(q4[:, 0:768], psHD0[:], AF.Abs)
            psHD1 = psum("hd1"); emit_half(psHD1, tH[1])
            nc.scalar.activation(q4[:, 768:1536], psHD1[:], AF.Abs)
            psC20 = psum("c20"); emit_half(psC20, tW[0])
            # c2 wants mosaic replication at the zeroed A.r/D.l halos (main+residual):
            nc.vector.tensor_tensor(psC20[:, 767:768], psC20[:, 767:768], P2[:, B_ + 768:B_ + 769], AL.add)
            nc.vector.tensor_tensor(psC20[:, 767:768], psC20[:, 767:768], P2r[:, RB + 768:RB + 769], AL.add)
            b2 = tmp("b2"); nc.vector.scalar_tensor_tensor(b2[:, 0:768], psC20[:], -0.5, hs[:, 0:768], AL.mult, AL.add)
            q2 = qf("q2"); nc.scalar.activation(q2[:, 0:768], psC20[:], AF.Abs)
            psC21 = psum("c21"); emit_half(psC21, tW[1])
            nc.vector.tensor_tensor(psC21[:, 0:1], psC21[:, 0:1], P2[:, C_ + 1:C_ + 2], AL.add)
            nc.vector.tensor_tensor(psC21[:, 0:1], psC21[:, 0:1], P2r[:, RC + 1:RC + 2], AL.add)
            nc.vector.scalar_tensor_tensor(b2[:, 768:1536], psC21[:], -0.5, hs[:, 768:1536], AL.mult, AL.add)
            nc.scalar.activation(q2[:, 768:1536], psC21[:], AF.Abs)

            clh = qf("clh"); nc.gpsimd.tensor_tensor(clh[:], q4[:], q2[:], AL.add)
            clv = qf("clv"); nc.gpsimd.tensor_tensor(clv[:], q5[:], q3[:], AL.add)
            dt = qf("dt"); nc.gpsimd.tensor_tensor(dt[:], clh[:], clv[:], AL.subtract)
            mk = msk("mk"); nc.vector.tensor_scalar(mk[:], dt[:], 0.0, None, AL.is_gt)

            tR = taps['RV']
            gint = pv(G2, G00 + 1, G11 + 1)
            nc.vector.tensor_scalar(gint, b2[:], 0.5, None, AL.mult)
            psRV0 = psum("rv0"); emit_half(psRV0, tR[0])
            nc.vector.copy_predicated(G2[:, G00 + 1:G00 + 769], mk[:, 0:768], psRV0[:])
            psRV1 = psum("rv1"); emit_half(psRV1, tR[1])
            nc.vector.copy_predicated(G2[:, G11 + 1:G11 + 769], mk[:, 768:1536], psRV1[:])

            # green halo cols (before PE reads G2)
            cc(G2[:, G00 + 769:G00 + 770], P2[:, B_ + 768:B_ + 769])  # g00.r = B[767]
            cc(G2[:, G11:G11 + 1], P2[:, C_ + 1:C_ + 2])              # g11.l = C[0]

            # green outputs: raw (0,1),(1,0) and computed (0,0),(1,1)
            nc.gpsimd.tensor_copy(pv(O, CG + 1, CG + 1536, step=2), pv(P2, B_ + 1, C_ + 1))
            nc.gpsimd.tensor_copy(pv(O, CG, CG + 1537, step=2), gint)

            # ---- hv fields ----
            tU = taps['U']
            psU0 = psum("u0"); emit_half(psU0, tU[0])
            psU1 = psum("u1"); emit_half(psU1, tU[1])

            t1h = tmp("t1h"); nc.vector.tensor_tensor(t1h[:], pv(P2, A_ + 1, D_), pv(P2, A_ + 2, D_ + 1), AL.add)
            t2h = tmp("t2h"); nc.vector.tensor_tensor(t2h[:], pv(G2, G00 + 1, G11), pv(G2, G00 + 2, G11 + 1), AL.add)
            uh = tmp("uh"); nc.vector.scalar_tensor_tensor(uh[:], t2h[:], -0.5, t1h[:], AL.mult, AL.add)

            hbc = tmp("hbc"); nc.vector.tensor_scalar(hbc[:], pv(P2, B_ + 1, C_ + 1), 0.5, None, AL.mult)
            # [C|B] pair is order-reversed in P2; compute halves separately into one tmp
            hcb = tmp("hcb")
            nc.vector.tensor_scalar(hcb[:, 0:768], P2[:, C_ + 1:C_ + 769], 0.5, None, AL.mult)
            nc.vector.tensor_scalar(hcb[:, 768:1536], P2[:, B_ + 1:B_ + 769], 0.5, None, AL.mult)

            nc.vector.scalar_tensor_tensor(pv(O, CR + 1, CB + 1536, step=2), uh[:], 0.5, hbc[:], AL.mult, AL.add)
            _d = pv(O, CR + 1536, CB + 1, step=2)
            _d0 = bass.AP(tensor=_d.tensor, offset=_d.offset, ap=[_d.ap[0], [2, 768]])
            _d1 = bass.AP(tensor=_d.tensor, offset=_d.offset + (CB + 1 - (CR + 1536)), ap=[_d.ap[0], [2, 768]])
            nc.vector.scalar_tensor_tensor(_d0, psU0[:], 0.5, hcb[:, 0:768], AL.mult, AL.add)
            nc.vector.scalar_tensor_tensor(_d1, psU1[:], 0.5, hcb[:, 768:1536], AL.mult, AL.add)

            # ---- chan (R11 | B00) ----
            tS = taps['S']
            psS0 = psum("s0"); emit_half(psS0, tS[0])
            nc.scalar.copy(P2[:, AU + 1:AU + 769], psS0[:])
            nc.vector.memset(P2[:, AU + 769:AU + 770], 0.0)
            psS1 = psum("s1"); emit_half(psS1, tS[1])
            nc.scalar.copy(P2[:, DD + 1:DD + 769], psS1[:])
            nc.vector.memset(P2[:, DD:DD + 1], 0.0)

            rp = tmp("rp"); nc.vector.tensor_tensor(rp[:], pv(P2, A_ + 1, DD), pv(P2, AU + 2, D_ + 1), AL.add)
            rn = tmp("rn"); nc.vector.tensor_tensor(rn[:], pv(P2, A_ + 2, DD + 1), pv(P2, AU + 1, D_), AL.add)

            tDM = taps['DM']; tDN = taps['DN']
            psDM0 = psum("dm0"); emit_half(psDM0, tDM[0])
            qdm = qf("qdm"); nc.scalar.activation(qdm[:, 0:768], psDM0[:], AF.Abs)
            psDM1 = psum("dm1"); emit_half(psDM1, tDM[1])
            nc.scalar.activation(qdm[:, 768:1536], psDM1[:], AF.Abs)
            psDN0 = psum("dn0"); emit_half(psDN0, tDN[0])
            qdn = qf("qdn"); nc.scalar.activation(qdn[:, 0:768], psDN0[:], AF.Abs)
            psDN1 = psum("dn1"); emit_half(psDN1, tDN[1])
            nc.scalar.activation(qdn[:, 768:1536], psDN1[:], AF.Abs)

            tG2_ = taps['CG2']; tG3_ = taps['CG3']
            psCG20 = psum("g20"); emit_half(psCG20, tG2_[0])
            qp = qf("qp"); nc.scalar.activation(qp[:, 0:768], psCG20[:], AF.Abs)
            cp2 = tmp("cp2"); nc.vector.scalar_tensor_tensor(cp2[:, 0:768], psCG20[:], -0.5, rp[:, 0:768], AL.mult, AL.add)
            psCG21 = psum("g21"); emit_half(psCG21, tG2_[1])
            nc.scalar.activation(qp[:, 768:1536], psCG21[:], AF.Abs)
            nc.vector.scalar_tensor_tensor(cp2[:, 768:1536], psCG21[:], -0.5, rp[:, 768:1536], AL.mult, AL.add)
            psCG30 = psum("g30"); emit_half(psCG30, tG3_[0])
            qn = qf("qn"); nc.scalar.activation(qn[:, 0:768], psCG30[:], AF.Abs)
            cn2 = tmp("cn2"); nc.vector.scalar_tensor_tensor(cn2[:, 0:768], psCG30[:], -0.5, rn[:, 0:768], AL.mult, AL.add)
            psCG31 = psum("g31"); emit_half(psCG31, tG3_[1])
            nc.scalar.activation(qn[:, 768:1536], psCG31[:], AF.Abs)
            nc.vector.scalar_tensor_tensor(cn2[:, 768:1536], psCG31[:], -0.5, rn[:, 768:1536], AL.mult, AL.add)
            clp = qf("clp"); nc.gpsimd.tensor_tensor(clp[:], qdm[:], qp[:], AL.add)
            cln = qf("cln"); nc.gpsimd.tensor_tensor(cln[:], qdn[:], qn[:], AL.add)
            dr = qf("dr"); nc.gpsimd.tensor_tensor(dr[:], clp[:], cln[:], AL.subtract)
            mr = msk("mr"); nc.vector.tensor_scalar(mr[:], dr[:], 0.0, None, AL.is_gt)
            cnh = tmp("cnh"); nc.vector.tensor_scalar(cnh[:], cn2[:], 0.5, None, AL.mult)
            odst = pv(O, CR + 1537, CB, step=2)
            nc.scalar.mul(odst, cp2[:], 0.5)
            nc.vector.copy_predicated(odst, mr[:], cnh[:])

            # ---- raw red/blue phases ----
            nc.gpsimd.tensor_copy(pv(O, CR, CB + 1537, step=2), pv(P2, A_ + 1, D_ + 1))

            # ---- output DMA (3 colors on 3 queues) ----
            if kind == 'l':
                p0, pn, row0 = 102, 24, 744
            else:
                p0, pn, row0 = 2, 124, r0
            nc.sync.dma_start(out_v[0, row0:row0 + pn, :], O[p0:p0 + pn, CR:CR + 3072])
            nc.scalar.dma_start(out_v[1, row0:row0 + pn, :], O[p0:p0 + pn, CG:CG + 3072])
            nc.gpsimd.dma_start(out_v[2, row0:row0 + pn, :], O[p0:p0 + pn, CB:CB + 3072])

    nc.compile()
    return nc


_NC_CACHE = None


def kernel(x: np.ndarray) -> np.ndarray:
    global _NC_CACHE
    if _NC_CACHE is None:
        _NC_CACHE = build_nc()
    xb = np.ascontiguousarray(x.astype(BFNP))
    xr = np.ascontiguousarray((x.astype(np.float32) - xb.astype(np.float32)).astype(BFNP))
    wb = np.ascontiguousarray(WBANDS)
    in_maps = [{"x": xb[i], "xr": xr[i], "wb": wb} for i in range(NCORES)]
    res = run_bass_kernel_spmd(_NC_CACHE, in_maps, list(range(NCORES)))
    return np.stack([res.results[i]["out"].astype(np.float32) for i in range(NCORES)], axis=0)


# revision 7
# speedup vs baseline: 1.0544x; 1.0392x over previous
"""Hamilton-Adams demosaic for Trainium2 — v2 (bf16, PE banded verticals, paired ops).

Input:  x [8, 4, 768, 768] f32  (quarter-res planes A=R, B=Gr, C=Gb, D=B)
Output: [8, 3, 1536, 1536] f32

All device compute in bf16 (host casts in/out).  Every gradient feeding a
directional-selection compare (vdif, c3, hdif, c2, dm, dn) runs on the Tensor
engine as a banded-stationary matmul accumulating BOTH the bf16 input and a
bf16 residual channel (xr = x - bf16(x)) in fp32 PSUM, so selection decisions
are effectively fp32-exact; only output values carry bf16 noise (l2 ~8e-3,
budget 2e-2).  Cross-partition (vertical) stencils and shifts are all PE
matmuls (W [128,128] per band, edge strips get modified bands + patch taps);
horizontal value ops run on DVE; |x| on Act; compares/copies on Pool.  The two
symmetric phases of each computation are fused into single wide ops via 3-dim
access patterns with a per-half column offset; PSUM results use 2-bank
single-half tiles on a 4-tag rotation for pipeline depth.

Tile layouts (halo col each side of every 770-wide plane region):
  P2  [128, 6*770] bf16: [au | A | B | C | D | dd]  (au=A row+1, dd=D row-1)
  P2r [128, 4*770+2] bf16: residual planes [A | B | C | D]
  G2  [128, 2*770] bf16: [g00 | g11]
  O   [128, 3*3072] bf16: interleaved full-res output rows (R | G | B)
"""
import sys
sys.path.insert(0, '/opt/trn_rl_repo')

from contextlib import ExitStack

import numpy as np
import ml_dtypes

import concourse.bass as bass
import concourse.bacc as bacc
import concourse.tile as tile
from concourse import mybir
from concourse.bass_utils import run_bass_kernel_spmd

F = mybir.dt.float32
BF = mybir.dt.bfloat16
U8 = mybir.dt.uint8
AL = mybir.AluOpType
AF = mybir.ActivationFunctionType
BFNP = ml_dtypes.bfloat16

H = 768
PW = 770
NCORES = 8
STRIPS = [0, 124, 248, 372, 496, 620, 644]

# P2 region base cols
AU, A_, B_, C_, D_, DD = 0, 770, 1540, 2310, 3080, 3850
# P2r (bf16 residual planes) region base cols — offset by 1 so taps with
# column delta -1/+1 on the first/last region stay inside the tile
RA, RB, RC, RD = 1, 771, 1541, 2311
# G2 region base cols
G00, G11 = 0, 770
# O color base cols
CR, CG, CB = 0, 3072, 6144

# ---------------- host-side band matrices ----------------
_mats: list[np.ndarray] = []
_mat_idx: dict = {}


def _W(taps=None, colfix=None):
    taps = taps or {}
    colfix = colfix or {}
    key = (tuple(sorted(taps.items())),
           tuple(sorted((m, tuple(sorted(f.items()))) for m, f in colfix.items())))
    if key in _mat_idx:
        return _mat_idx[key]
    M = np.zeros((128, 128), np.float32)
    for m in range(128):
        if m in colfix:
            for k, w in colfix[m].items():
                M[k, m] = w
        else:
            for d, w in taps.items():
                k = m + d
                if 0 <= k < 128:
                    M[k, m] = w
    idx = len(_mats)
    _mats.append(M)
    _mat_idx[key] = idx
    return idx


def _tap_table(kind):
    """kind in 'f' (first strip), 'm' (mid), 'l' (last).
    Returns result -> (half0 taps, half1 taps); tap = (Widx, src_region).
    src_region is a P2 or G2 base col; G-sourced taps marked by ('G', base)."""
    f, l = kind == 'f', kind == 'l'

    def FF(fix):
        return {0: {}, 1: {}, **fix}

    def LL(fix):
        return {**fix, 126: {}, 127: {}}

    def BW(taps, ffix=None, lfix=None):
        cf = None
        if f and ffix is not None:
            cf = FF(ffix)
        if l and lfix is not None:
            cf = LL(lfix)
        return _W(taps, cf)

    P22 = _W({}, {2: {2: 1.0}})
    P125 = _W({}, {125: {125: 1.0}})
    P125n = _W({}, {125: {125: -1.0}})
    P = lambda b: ('P', b)
    G = lambda b: ('G', b)
    R = lambda b: ('R', b)
    RES = {A_: RA, B_: RB, C_: RC, D_: RD}

    def with_res(taps):
        # duplicate every P-plane tap onto the residual tile (same W, same delta)
        out = list(taps)
        for wi, (sk, sb), dlt in taps:
            if sk == 'P':
                out.append((wi, R(RES[sb]), dlt))
        return out

    I1 = _W({0: 1.0})
    NegI = _W({0: -1.0})
    Neg2I = _W({0: -2.0})
    D1 = _W({1: 1.0}, LL({125: {}}) if l else None)   # down-shift; zero bottom edge
    U1 = _W({-1: 1.0}, FF({2: {}}) if f else None)    # up-shift; zero top edge
    NegU1 = _W({-1: -1.0}, FF({2: {}}) if f else None)

    T = {}
    # VD: vdif00 = C[i-1]-C[i]  |  vdif11 = B[i]-B[i+1]   (exact: + residual taps)
    T['VD'] = (
        with_res([(BW({-1: 1, 0: -1}, ffix={2: {2: -1}}), P(C_), 0)] + ([(P22, P(A_), 0)] if f else [])),
        with_res([(BW({0: 1, 1: -1}, lfix={125: {125: 1}}), P(B_), 0)] + ([(P125n, P(D_), 0)] if l else [])),
    )
    # C3: A[i-1]-2A[i]+A[i+1]  |  D[i-1]-2D[i]+D[i+1]
    T['C3'] = (
        with_res([(BW({-1: 1, 0: -2, 1: 1}, ffix={2: {2: -1, 3: 1}}, lfix={125: {124: 1, 125: -2}}), P(A_), 0)]
                 + ([(P125, P(C_), 0)] if l else [])),
        with_res([(BW({-1: 1, 0: -2, 1: 1}, ffix={2: {2: -2, 3: 1}}, lfix={125: {124: 1, 125: -1}}), P(D_), 0)]
                 + ([(P22, P(B_), 0)] if f else [])),
    )
    # HD: hdif00 = B[j-1]-B[j]  |  hdif11 = C[j]-C[j+1]   (horizontal, via col deltas)
    T['HD'] = (
        with_res([(I1, P(B_), -1), (NegI, P(B_), 0)]),
        with_res([(I1, P(C_), 0), (NegI, P(C_), 1)]),
    )
    # C2: A[j-1]-2A[j]+A[j+1]  |  D[j-1]-2D[j]+D[j+1]  (edge cols fixed on psum)
    T['C2'] = (
        with_res([(I1, P(A_), -1), (Neg2I, P(A_), 0), (I1, P(A_), 1)]),
        with_res([(I1, P(D_), -1), (Neg2I, P(D_), 0), (I1, P(D_), 1)]),
    )
    # RV: rawv = 0.5*vsum - 0.25*c3 (the vertical green candidate, final scale)
    T['RV'] = (
        [(BW({-1: .5, 0: .5}, ffix={2: {2: .5}}, lfix={125: {124: .5, 125: .25}}), P(C_), 0),
         (BW({-1: -.25, 0: .5, 1: -.25}, ffix={2: {2: .75, 3: -.25}},
             lfix={125: {124: -.25, 125: .5}}), P(A_), 0)],
        [(BW({0: .5, 1: .5}, ffix={2: {2: .25, 3: .5}}, lfix={125: {125: .5}}), P(B_), 0),
         (BW({-1: -.25, 0: .5, 1: -.25}, ffix={2: {2: .5, 3: -.25}},
             lfix={125: {124: -.25, 125: .75}}), P(D_), 0)],
    )
    # U = t1 - 0.5*t2 for the vertical hv pair (R10 | B01):
    #   R10: (A[i]+A[i+1]) - 0.5*(g00[i]+g00[i+1])   (bottom edge: A only / +C patch)
    #   B01: (D[i-1]+D[i]) - 0.5*(g11[i-1]+g11[i])   (top edge: D only / +B patch)
    P125h = _W({}, {125: {125: -0.5}})
    P22h = _W({}, {2: {2: -0.5}})
    T['U'] = (
        [(BW({0: 1, 1: 1}, lfix={125: {125: 1}}), P(A_), 0),
         (BW({0: -.5, 1: -.5}, lfix={125: {125: -.5}}), G(G00), 0)]
        + ([(P125h, P(C_), 0)] if l else []),
        [(BW({-1: 1, 0: 1}, ffix={2: {2: 1}}), P(D_), 0),
         (BW({-1: -.5, 0: -.5}, ffix={2: {2: -.5}}), G(G11), 0)]
        + ([(P22h, P(B_), 0)] if f else []),
    )
    # S: au = A[i+1] (last: 0)  |  dd = D[i-1] (first: 0)
    T['S'] = (
        [(D1, P(A_), 0)],
        [(U1, P(D_), 0)],
    )
    # DM: |R11| au[j+1]-A[j] = A[i+1,j+1]-A[i,j]  |  |B00| D[i,j]-D[i-1,j-1]
    T['DM'] = (
        [(D1, P(A_), 1), (NegI, P(A_), 0)],
        [(I1, P(D_), 0), (NegU1, P(D_), -1)],
    )
    # DN: R11: A[i+1,j]-A[i,j+1]  |  B00: D[i,j-1]-D[i-1,j]
    T['DN'] = (
        [(D1, P(A_), 0), (NegI, P(A_), 1)],
        [(I1, P(D_), -1), (NegU1, P(D_), 0)],
    )
    # CG2 = sp - 2*gnear;  sp_R11 = g00[i,j]+g00[i+1,j+1], sp_B00 = g11[i-1,j-1]+g11[i,j]
    T['CG2'] = (
        [(I1, G(G00), 0), (D1, G(G00), 1)] + ([(P125, P(C_), 1)] if l else [])
        + [(Neg2I, G(G11), 0)],
        [(U1, G(G11), -1)] + ([(P22, P(B_), -1)] if f else [])
        + [(I1, G(G11), 0), (Neg2I, G(G00), 0)],
    )
    # CG3 = sn - 2*gnear;  sn_R11 = g00[i,j+1]+g00[i+1,j], sn_B00 = g11[i-1,j]+g11[i,j-1]
    T['CG3'] = (
        [(I1, G(G00), 1), (D1, G(G00), 0)] + ([(P125, P(C_), 0)] if l else [])
        + [(Neg2I, G(G11), 0)],
        [(U1, G(G11), 0)] + ([(P22, P(B_), 0)] if f else [])
        + [(I1, G(G11), -1), (Neg2I, G(G00), 0)],
    )
    return T


_TAPS = {k: _tap_table(k) for k in ('f', 'm', 'l')}
NW = len(_mats)
WBANDS = np.concatenate([m.astype(BFNP) for m in _mats], axis=1)  # [128, NW*128]


def pv(t, o0, o1, w=768, step=1):
    """Pair view: [128][2 halves at col offsets o0/o1][w cols at `step`]."""
    b = t[:]
    assert o1 > o0, (o0, o1)
    return bass.AP(tensor=b.tensor, offset=b.offset + o0,
                   ap=[[b.ap[0][0], 128], [o1 - o0, 2], [step, w]])


def build_nc():
    nc = bacc.Bacc("TRN2", target_bir_lowering=False, debug=False, num_devices=NCORES)
    x_in = nc.declare_dram_parameter("x", [4, H, H], BF, isOutput=False)
    xr_in = nc.declare_dram_parameter("xr", [4, H, H], BF, isOutput=False)
    wb_in = nc.declare_dram_parameter("wb", [128, NW * 128], BF, isOutput=False)
    out = nc.declare_dram_parameter("out", [3, 2 * H, 2 * H], BF, isOutput=True)
    out_v = out[:].rearrange("c (r two) w -> c r (two w)", two=2)
    x_r = x_in[:].rearrange("c r w -> r c w")
    xr_r = xr_in[:].rearrange("c r w -> r c w")

    with tile.TileContext(nc) as tc, ExitStack() as ctx:
        wpool = ctx.enter_context(tc.tile_pool(name="wpool", bufs=1))
        pl = ctx.enter_context(tc.tile_pool(name="planes", bufs=2))
        gp = ctx.enter_context(tc.tile_pool(name="greens", bufs=2))
        tp = ctx.enter_context(tc.tile_pool(name="tmps", bufs=16))
        qp_ = ctx.enter_context(tc.tile_pool(name="qtmps", bufs=6))
        mp = ctx.enter_context(tc.tile_pool(name="msks", bufs=2))
        op = ctx.enter_context(tc.tile_pool(name="outs", bufs=2))
        pp = ctx.enter_context(tc.tile_pool(name="psums", bufs=1, space="PSUM"))

        WS = wpool.tile([128, NW * 128], BF, tag="ws", name="ws")
        nc.sync.dma_start(WS[:], wb_in[:])

        for si, r0 in enumerate(STRIPS):
            kind = 'f' if si == 0 else ('l' if si == len(STRIPS) - 1 else 'm')
            taps = _TAPS[kind]
            base = r0 - 2

            psum_ctr = [0]
            P2 = pl.tile([128, 6 * PW], BF, tag="P2", name=f"P2_{si}")
            P2r = pl.tile([128, 4 * PW + 2], BF, tag="P2r", name=f"P2r_{si}")
            G2 = gp.tile([128, 2 * PW], BF, tag="G2", name=f"G2_{si}")
            O = op.tile([128, 3 * 3072], BF, tag="O", name=f"O_{si}")

            def tmp(tag):
                return tp.tile([128, 1536], BF, tag="t", name=f"{tag}_{si}")

            def qf(tag):
                return qp_.tile([128, 1536], F, tag="q", name=f"{tag}_{si}")

            def msk(tag):
                return mp.tile([128, 1536], U8, tag=tag, name=f"{tag}_{si}")

            def psum(tag):
                i = psum_ctr[0] % 4
                psum_ctr[0] += 1
                return pp.tile([128, 768], F, tag=f"u{i}", name=f"{tag}_{si}")

            def emit_half(ps, tl):
                for (c0, c1) in ((0, 512), (512, 768)):
                    n = len(tl)
                    for i, (wi, (sk, sb), dlt) in enumerate(tl):
                        src = {'P': P2, 'G': G2, 'R': P2r}[sk]
                        nc.tensor.matmul(
                            ps[:, c0:c1],
                            WS[:, wi * 128:(wi + 1) * 128],
                            src[:, sb + 1 + dlt + c0:sb + 1 + dlt + c1],
                            start=(i == 0), stop=(i == n - 1))

            # ---- input DMA (planes A..D at P2 cols 770..3849) ----
            # compute-engine partition starts must be 32-aligned: memset a whole
            # 32-lane block first, then let the DMA overwrite the loaded lanes.
            if kind == 'f':
                nc.gpsimd.memset(P2[0:32, :], 0.0)
                nc.gpsimd.memset(P2r[0:32, :], 0.0)
            if kind == 'l':
                nc.gpsimd.memset(P2[96:128, :], 0.0)
                nc.gpsimd.memset(P2r[96:128, :], 0.0)
            clo, chi = max(base, 0), min(base + 128, H)
            Pr = P2[:].rearrange("p (six w) -> p six w", six=6)
            nc.sync.dma_start(Pr[clo - base: chi - base, 1:5, 1:1 + H], x_r[clo:chi, :, :])
            _b = P2r[clo - base: chi - base, :]
            Prr = bass.AP(tensor=_b.tensor, offset=_b.offset + RA + 1,
                          ap=[_b.ap[0], [PW, 4], [1, H]])
            nc.sync.dma_start(Prr, xr_r[clo:chi, :, :])

            # ---- plane halo cols ----
            # A.r and D.l are ZERO (masked-channel replication) — the two wide
            # ops that instead need the mosaic replication (c2 pair) get tiny
            # single-column fixups below.
            cc = nc.vector.tensor_copy
            cc(P2[:, A_:A_ + 1], P2[:, A_ + 1:A_ + 2])          # A.l = A[0]
            cc(P2[:, B_:B_ + 1], P2[:, A_ + 1:A_ + 2])          # B.l = A[0]
            nc.vector.memset(P2[:, D_:D_ + 1], 0.0)             # D.l = 0
            nc.vector.memset(P2[:, A_ + 769:A_ + 770], 0.0)     # A.r = 0
            cc(P2[:, C_ + 769:C_ + 770], P2[:, D_ + 768:D_ + 769])  # C.r = D[767]
            cc(P2[:, D_ + 769:D_ + 770], P2[:, D_ + 768:D_ + 769])  # D.r = D[767]
            cc(P2r[:, RA:RA + 1], P2r[:, RA + 1:RA + 2])
            cc(P2r[:, RB:RB + 1], P2r[:, RA + 1:RA + 2])
            nc.vector.memset(P2r[:, RD:RD + 1], 0.0)
            nc.vector.memset(P2r[:, RA + 769:RA + 770], 0.0)
            cc(P2r[:, RC + 769:RC + 770], P2r[:, RD + 768:RD + 769])
            cc(P2r[:, RD + 769:RD + 770], P2r[:, RD + 768:RD + 769])

            # ---- green: PE verticals (consumers emitted in psum-alloc order
            # so the in-order ACT queue can never cycle with PE buffer reuse) ----
            tV = taps['VD']; tC = taps['C3']; tH = taps['HD']; tW = taps['C2']
            psVD0 = psum("vd0"); emit_half(psVD0, tV[0])
            q5 = qf("q5"); nc.scalar.activation(q5[:, 0:768], psVD0[:], AF.Abs)
            psVD1 = psum("vd1"); emit_half(psVD1, tV[1])
            nc.scalar.activation(q5[:, 768:1536], psVD1[:], AF.Abs)
            psC30 = psum("c30"); emit_half(psC30, tC[0])
            q3 = qf("q3"); nc.scalar.activation(q3[:, 0:768], psC30[:], AF.Abs)
            psC31 = psum("c31"); emit_half(psC31, tC[1])
            nc.scalar.activation(q3[:, 768:1536], psC31[:], AF.Abs)

            # ---- green: horizontals ----
            hs = tmp("hs"); nc.vector.tensor_tensor(hs[:], pv(P2, B_, C_ + 1), pv(P2, B_ + 1, C_ + 2), AL.add)
            psHD0 = psum("hd0"); emit_half(psHD0, tH[0])
            q4 = qf("q4"); nc.scalar.activation(q4[:, 0:768], psHD0[:], AF.Abs)
            psHD1 = psum("hd1"); emit_half(psHD1, tH[1])
            nc.scalar.activation(q4[:, 768:1536], psHD1[:], AF.Abs)
            psC20 = psum("c20"); emit_half(psC20, tW[0])
            # c2 wants mosaic replication at the zeroed A.r/D.l halos (main+residual):
            nc.vector.tensor_tensor(psC20[:, 767:768], psC20[:, 767:768], P2[:, B_ + 768:B_ + 769], AL.add)
            nc.vector.tensor_tensor(psC20[:, 767:768], psC20[:, 767:768], P2r[:, RB + 768:RB + 769], AL.add)
            b2 = tmp("b2"); nc.vector.scalar_tensor_tensor(b2[:, 0:768], psC20[:], -0.5, hs[:, 0:768], AL.mult, AL.add)
            q2 = qf("q2"); nc.scalar.activation(q2[:, 0:768], psC20[:], AF.Abs)
            psC21 = psum("c21"); emit_half(psC21, tW[1])
            nc.vector.tensor_tensor(psC21[:, 0:1], psC21[:, 0:1], P2[:, C_ + 1:C_ + 2], AL.add)
            nc.vector.tensor_tensor(psC21[:, 0:1], psC21[:, 0:1], P2r[:, RC + 1:RC + 2], AL.add)
            nc.vector.scalar_tensor_tensor(b2[:, 768:1536], psC21[:], -0.5, hs[:, 768:1536], AL.mult, AL.add)
            nc.scalar.activation(q2[:, 768:1536], psC21[:], AF.Abs)

            clh = qf("clh"); nc.gpsimd.tensor_tensor(clh[:], q4[:], q2[:], AL.add)
            clv = qf("clv"); nc.gpsimd.tensor_tensor(clv[:], q5[:], q3[:], AL.add)
            dt = qf("dt"); nc.gpsimd.tensor_tensor(dt[:], clh[:], clv[:], AL.subtract)
            mk = msk("mk"); nc.vector.tensor_scalar(mk[:], dt[:], 0.0, None, AL.is_gt)

            tR = taps['RV']
            gint = pv(G2, G00 + 1, G11 + 1)
            nc.vector.tensor_scalar(gint, b2[:], 0.5, None, AL.mult)
            psRV0 = psum("rv0"); emit_half(psRV0, tR[0])
            nc.vector.copy_predicated(G2[:, G00 + 1:G00 + 769], mk[:, 0:768], psRV0[:])
            psRV1 = psum("rv1"); emit_half(psRV1, tR[1])
            nc.vector.copy_predicated(G2[:, G11 + 1:G11 + 769], mk[:, 768:1536], psRV1[:])

            # green halo cols (before PE reads G2)
            cc(G2[:, G00 + 769:G00 + 770], P2[:, B_ + 768:B_ + 769])  # g00.r = B[767]
            cc(G2[:, G11:G11 + 1], P2[:, C_ + 1:C_ + 2])              # g11.l = C[0]

            # green outputs: raw (0,1),(1,0) and computed (0,0),(1,1)
            nc.gpsimd.tensor_copy(pv(O, CG + 1, CG + 1536, step=2), pv(P2, B_ + 1, C_ + 1))
            nc.gpsimd.tensor_copy(pv(O, CG, CG + 1537, step=2), gint)

            # ---- hv fields ----
            tU = taps['U']
            psU0 = psum("u0"); emit_half(psU0, tU[0])
            psU1 = psum("u1"); emit_half(psU1, tU[1])

            t1h = tmp("t1h"); nc.vector.tensor_tensor(t1h[:], pv(P2, A_ + 1, D_), pv(P2, A_ + 2, D_ + 1), AL.add)
            t2h = tmp("t2h"); nc.vector.tensor_tensor(t2h[:], pv(G2, G00 + 1, G11), pv(G2, G00 + 2, G11 + 1), AL.add)
            uh = tmp("uh"); nc.vector.scalar_tensor_tensor(uh[:], t2h[:], -0.5, t1h[:], AL.mult, AL.add)

            # hbc halves are 0.5*B | 0.5*C; the R10/B01 ops below use them swapped
            hbc = tmp("hbc"); nc.vector.tensor_scalar(hbc[:], pv(P2, B_ + 1, C_ + 1), 0.5, None, AL.mult)

            nc.vector.scalar_tensor_tensor(pv(O, CR + 1, CB + 1536, step=2), uh[:], 0.5, hbc[:], AL.mult, AL.add)
            _d = pv(O, CR + 1536, CB + 1, step=2)
            _d0 = bass.AP(tensor=_d.tensor, offset=_d.offset, ap=[_d.ap[0], [2, 768]])
            _d1 = bass.AP(tensor=_d.tensor, offset=_d.offset + (CB + 1 - (CR + 1536)), ap=[_d.ap[0], [2, 768]])
            nc.vector.scalar_tensor_tensor(_d0, psU0[:], 0.5, hbc[:, 768:1536], AL.mult, AL.add)
            nc.vector.scalar_tensor_tensor(_d1, psU1[:], 0.5, hbc[:, 0:768], AL.mult, AL.add)

            # ---- chan (R11 | B00) ----
            tS = taps['S']
            psS0 = psum("s0"); emit_half(psS0, tS[0])
            nc.scalar.copy(P2[:, AU + 1:AU + 769], psS0[:])
            nc.vector.memset(P2[:, AU + 769:AU + 770], 0.0)
            psS1 = psum("s1"); emit_half(psS1, tS[1])
            nc.scalar.copy(P2[:, DD + 1:DD + 769], psS1[:])
            nc.vector.memset(P2[:, DD:DD + 1], 0.0)

            rp = tmp("rp"); nc.gpsimd.tensor_tensor(rp[:], pv(P2, A_ + 1, DD), pv(P2, AU + 2, D_ + 1), AL.add)
            rn = tmp("rn"); nc.gpsimd.tensor_tensor(rn[:], pv(P2, A_ + 2, DD + 1), pv(P2, AU + 1, D_), AL.add)

            tDM = taps['DM']; tDN = taps['DN']
            psDM0 = psum("dm0"); emit_half(psDM0, tDM[0])
            qdm = qf("qdm"); nc.scalar.activation(qdm[:, 0:768], psDM0[:], AF.Abs)
            psDM1 = psum("dm1"); emit_half(psDM1, tDM[1])
            nc.scalar.activation(qdm[:, 768:1536], psDM1[:], AF.Abs)
            psDN0 = psum("dn0"); emit_half(psDN0, tDN[0])
            qdn = qf("qdn"); nc.scalar.activation(qdn[:, 0:768], psDN0[:], AF.Abs)
            psDN1 = psum("dn1"); emit_half(psDN1, tDN[1])
            nc.scalar.activation(qdn[:, 768:1536], psDN1[:], AF.Abs)

            tG2_ = taps['CG2']; tG3_ = taps['CG3']
            psCG20 = psum("g20"); emit_half(psCG20, tG2_[0])
            qp = qf("qp"); nc.scalar.activation(qp[:, 0:768], psCG20[:], AF.Abs)
            cp2 = tmp("cp2"); nc.vector.scalar_tensor_tensor(cp2[:, 0:768], psCG20[:], -0.5, rp[:, 0:768], AL.mult, AL.add)
            psCG21 = psum("g21"); emit_half(psCG21, tG2_[1])
            nc.scalar.activation(qp[:, 768:1536], psCG21[:], AF.Abs)
            nc.vector.scalar_tensor_tensor(cp2[:, 768:1536], psCG21[:], -0.5, rp[:, 768:1536], AL.mult, AL.add)
            psCG30 = psum("g30"); emit_half(psCG30, tG3_[0])
            qn = qf("qn"); nc.scalar.activation(qn[:, 0:768], psCG30[:], AF.Abs)
            cn2 = tmp("cn2"); nc.vector.scalar_tensor_tensor(cn2[:, 0:768], psCG30[:], -0.5, rn[:, 0:768], AL.mult, AL.add)
            psCG31 = psum("g31"); emit_half(psCG31, tG3_[1])
            nc.scalar.activation(qn[:, 768:1536], psCG31[:], AF.Abs)
            nc.vector.scalar_tensor_tensor(cn2[:, 768:1536], psCG31[:], -0.5, rn[:, 768:1536], AL.mult, AL.add)
            clp = qf("clp"); nc.gpsimd.tensor_tensor(clp[:], qdm[:], qp[:], AL.add)
            cln = qf("cln"); nc.gpsimd.tensor_tensor(cln[:], qdn[:], qn[:], AL.add)
            dr = qf("dr"); nc.gpsimd.tensor_tensor(dr[:], clp[:], cln[:], AL.subtract)
            mr = msk("mr"); nc.vector.tensor_scalar(mr[:], dr[:], 0.0, None, AL.is_gt)
            cnh = tmp("cnh"); nc.vector.tensor_scalar(cnh[:], cn2[:], 0.5, None, AL.mult)
            odst = pv(O, CR + 1537, CB, step=2)
            nc.scalar.mul(odst, cp2[:], 0.5)
            nc.vector.copy_predicated(odst, mr[:], cnh[:])

            # ---- raw red/blue phases ----
            nc.gpsimd.tensor_copy(pv(O, CR, CB + 1537, step=2), pv(P2, A_ + 1, D_ + 1))

            # ---- output DMA (3 colors on 3 queues) ----
            if kind == 'l':
                p0, pn, row0 = 102, 24, 744
            else:
                p0, pn, row0 = 2, 124, r0
            nc.sync.dma_start(out_v[0, row0:row0 + pn, :], O[p0:p0 + pn, CR:CR + 3072])
            nc.scalar.dma_start(out_v[1, row0:row0 + pn, :], O[p0:p0 + pn, CG:CG + 3072])
            nc.gpsimd.dma_start(out_v[2, row0:row0 + pn, :], O[p0:p0 + pn, CB:CB + 3072])

    nc.compile()
    return nc


_NC_CACHE = None


def kernel(x: np.ndarray) -> np.ndarray:
    global _NC_CACHE
    if _NC_CACHE is None:
        _NC_CACHE = build_nc()
    xb = np.ascontiguousarray(x.astype(BFNP))
    xr = np.ascontiguousarray((x.astype(np.float32) - xb.astype(np.float32)).astype(BFNP))
    wb = np.ascontiguousarray(WBANDS)
    in_maps = [{"x": xb[i], "xr": xr[i], "wb": wb} for i in range(NCORES)]
    res = run_bass_kernel_spmd(_NC_CACHE, in_maps, list(range(NCORES)))
    return np.stack([res.results[i]["out"].astype(np.float32) for i in range(NCORES)], axis=0)


# revision 8
# speedup vs baseline: 1.1664x; 1.1062x over previous
"""Hamilton-Adams demosaic for Trainium2 — v2 (bf16, PE banded verticals, paired ops).

Input:  x [8, 4, 768, 768] f32  (quarter-res planes A=R, B=Gr, C=Gb, D=B)
Output: [8, 3, 1536, 1536] f32

All device compute in bf16 (host casts in/out).  Every gradient feeding a
directional-selection compare (vdif, c3, hdif, c2, dm, dn) runs on the Tensor
engine as a banded-stationary matmul accumulating BOTH the bf16 input and a
bf16 residual channel (xr = x - bf16(x)) in fp32 PSUM, so selection decisions
are effectively fp32-exact; only output values carry bf16 noise (l2 ~8e-3,
budget 2e-2).  Cross-partition (vertical) stencils and shifts are all PE
matmuls (W [128,128] per band, edge strips get modified bands + patch taps);
horizontal value ops run on DVE; |x| on Act; compares/copies on Pool.  The two
symmetric phases of each computation are fused into single wide ops via 3-dim
access patterns with a per-half column offset; PSUM results use 2-bank
single-half tiles on a 4-tag rotation for pipeline depth.

Tile layouts (halo col each side of every 770-wide plane region):
  P2  [128, 6*770] bf16: [au | A | B | C | D | dd]  (au=A row+1, dd=D row-1)
  P2r [128, 4*770+2] bf16: residual planes [A | B | C | D]
  G2  [128, 2*770] bf16: [g00 | g11]
  O   [128, 3*3072] bf16: interleaved full-res output rows (R | G | B)
"""
import sys
sys.path.insert(0, '/opt/trn_rl_repo')

from contextlib import ExitStack

import numpy as np
import ml_dtypes

import concourse.bass as bass
import concourse.bacc as bacc
import concourse.tile as tile
from concourse import mybir
from concourse.bass_utils import run_bass_kernel_spmd

F = mybir.dt.float32
BF = mybir.dt.bfloat16
U8 = mybir.dt.uint8
AL = mybir.AluOpType
AF = mybir.ActivationFunctionType
BFNP = ml_dtypes.bfloat16

H = 768
PW = 770
NCORES = 8
STRIPS = [0, 124, 248, 372, 496, 620, 644]

# P2 region base cols
AU, A_, B_, C_, D_, DD = 0, 770, 1540, 2310, 3080, 3850
# P2r (bf16 residual planes) region base cols — offset by 1 so taps with
# column delta -1/+1 on the first/last region stay inside the tile
RA, RB, RC, RD = 1, 771, 1541, 2311
# G2 region base cols
G00, G11 = 0, 770
# O color base cols
CR, CG, CB = 0, 3072, 6144

# ---------------- host-side band matrices ----------------
_mats: list[np.ndarray] = []
_mat_idx: dict = {}


def _W(taps=None, colfix=None):
    taps = taps or {}
    colfix = colfix or {}
    key = (tuple(sorted(taps.items())),
           tuple(sorted((m, tuple(sorted(f.items()))) for m, f in colfix.items())))
    if key in _mat_idx:
        return _mat_idx[key]
    M = np.zeros((128, 128), np.float32)
    for m in range(128):
        if m in colfix:
            for k, w in colfix[m].items():
                M[k, m] = w
        else:
            for d, w in taps.items():
                k = m + d
                if 0 <= k < 128:
                    M[k, m] = w
    idx = len(_mats)
    _mats.append(M)
    _mat_idx[key] = idx
    return idx


def _tap_table(kind):
    """kind in 'f' (first strip), 'm' (mid), 'l' (last).
    Returns result -> (half0 taps, half1 taps); tap = (Widx, src_region).
    src_region is a P2 or G2 base col; G-sourced taps marked by ('G', base)."""
    f, l = kind == 'f', kind == 'l'

    def FF(fix):
        return {0: {}, 1: {}, **fix}

    def LL(fix):
        return {**fix, 126: {}, 127: {}}

    def BW(taps, ffix=None, lfix=None):
        cf = None
        if f and ffix is not None:
            cf = FF(ffix)
        if l and lfix is not None:
            cf = LL(lfix)
        return _W(taps, cf)

    P22 = _W({}, {2: {2: 1.0}})
    P125 = _W({}, {125: {125: 1.0}})
    P125n = _W({}, {125: {125: -1.0}})
    P = lambda b: ('P', b)
    G = lambda b: ('G', b)
    R = lambda b: ('R', b)
    RES = {A_: RA, B_: RB, C_: RC, D_: RD}

    def with_res(taps):
        # duplicate every P-plane tap onto the residual tile (same W, same delta)
        out = list(taps)
        for wi, (sk, sb), dlt in taps:
            if sk == 'P':
                out.append((wi, R(RES[sb]), dlt))
        return out

    I1 = _W({0: 1.0})
    NegI = _W({0: -1.0})
    Neg2I = _W({0: -2.0})
    D1 = _W({1: 1.0}, LL({125: {}}) if l else None)   # down-shift; zero bottom edge
    U1 = _W({-1: 1.0}, FF({2: {}}) if f else None)    # up-shift; zero top edge
    NegU1 = _W({-1: -1.0}, FF({2: {}}) if f else None)

    T = {}
    # VD: vdif00 = C[i-1]-C[i]  |  vdif11 = B[i]-B[i+1]   (exact: + residual taps)
    T['VD'] = (
        with_res([(BW({-1: 1, 0: -1}, ffix={2: {2: -1}}), P(C_), 0)] + ([(P22, P(A_), 0)] if f else [])),
        with_res([(BW({0: 1, 1: -1}, lfix={125: {125: 1}}), P(B_), 0)] + ([(P125n, P(D_), 0)] if l else [])),
    )
    # C3: A[i-1]-2A[i]+A[i+1]  |  D[i-1]-2D[i]+D[i+1]
    T['C3'] = (
        with_res([(BW({-1: 1, 0: -2, 1: 1}, ffix={2: {2: -1, 3: 1}}, lfix={125: {124: 1, 125: -2}}), P(A_), 0)]
                 + ([(P125, P(C_), 0)] if l else [])),
        with_res([(BW({-1: 1, 0: -2, 1: 1}, ffix={2: {2: -2, 3: 1}}, lfix={125: {124: 1, 125: -1}}), P(D_), 0)]
                 + ([(P22, P(B_), 0)] if f else [])),
    )
    # HD: hdif00 = B[j-1]-B[j]  |  hdif11 = C[j]-C[j+1]   (horizontal, via col deltas)
    T['HD'] = (
        with_res([(I1, P(B_), -1), (NegI, P(B_), 0)]),
        with_res([(I1, P(C_), 0), (NegI, P(C_), 1)]),
    )
    # C2: A[j-1]-2A[j]+A[j+1]  |  D[j-1]-2D[j]+D[j+1]  (edge cols fixed on psum)
    T['C2'] = (
        with_res([(I1, P(A_), -1), (Neg2I, P(A_), 0), (I1, P(A_), 1)]),
        with_res([(I1, P(D_), -1), (Neg2I, P(D_), 0), (I1, P(D_), 1)]),
    )
    # RV: rawv = 0.5*vsum - 0.25*c3 (the vertical green candidate, final scale)
    T['RV'] = (
        [(BW({-1: .5, 0: .5}, ffix={2: {2: .5}}, lfix={125: {124: .5, 125: .25}}), P(C_), 0),
         (BW({-1: -.25, 0: .5, 1: -.25}, ffix={2: {2: .75, 3: -.25}},
             lfix={125: {124: -.25, 125: .5}}), P(A_), 0)],
        [(BW({0: .5, 1: .5}, ffix={2: {2: .25, 3: .5}}, lfix={125: {125: .5}}), P(B_), 0),
         (BW({-1: -.25, 0: .5, 1: -.25}, ffix={2: {2: .5, 3: -.25}},
             lfix={125: {124: -.25, 125: .75}}), P(D_), 0)],
    )
    # U = t1 - 0.5*t2 for the vertical hv pair (R10 | B01):
    #   R10: (A[i]+A[i+1]) - 0.5*(g00[i]+g00[i+1])   (bottom edge: A only / +C patch)
    #   B01: (D[i-1]+D[i]) - 0.5*(g11[i-1]+g11[i])   (top edge: D only / +B patch)
    P125h = _W({}, {125: {125: -0.5}})
    P22h = _W({}, {2: {2: -0.5}})
    T['U'] = (
        [(BW({0: 1, 1: 1}, lfix={125: {125: 1}}), P(A_), 0),
         (BW({0: -.5, 1: -.5}, lfix={125: {125: -.5}}), G(G00), 0)]
        + ([(P125h, P(C_), 0)] if l else []),
        [(BW({-1: 1, 0: 1}, ffix={2: {2: 1}}), P(D_), 0),
         (BW({-1: -.5, 0: -.5}, ffix={2: {2: -.5}}), G(G11), 0)]
        + ([(P22h, P(B_), 0)] if f else []),
    )
    # S: au = A[i+1] (last: 0)  |  dd = D[i-1] (first: 0)
    T['S'] = (
        [(D1, P(A_), 0)],
        [(U1, P(D_), 0)],
    )
    # DM: |R11| au[j+1]-A[j] = A[i+1,j+1]-A[i,j]  |  |B00| D[i,j]-D[i-1,j-1]
    T['DM'] = (
        [(D1, P(A_), 1), (NegI, P(A_), 0)],
        [(I1, P(D_), 0), (NegU1, P(D_), -1)],
    )
    # DN: R11: A[i+1,j]-A[i,j+1]  |  B00: D[i,j-1]-D[i-1,j]
    T['DN'] = (
        [(D1, P(A_), 0), (NegI, P(A_), 1)],
        [(I1, P(D_), -1), (NegU1, P(D_), 0)],
    )
    # CG2 = sp - 2*gnear;  sp_R11 = g00[i,j]+g00[i+1,j+1], sp_B00 = g11[i-1,j-1]+g11[i,j]
    T['CG2'] = (
        [(I1, G(G00), 0), (D1, G(G00), 1)] + ([(P125, P(C_), 1)] if l else [])
        + [(Neg2I, G(G11), 0)],
        [(U1, G(G11), -1)] + ([(P22, P(B_), -1)] if f else [])
        + [(I1, G(G11), 0), (Neg2I, G(G00), 0)],
    )
    # CG3 = sn - 2*gnear;  sn_R11 = g00[i,j+1]+g00[i+1,j], sn_B00 = g11[i-1,j]+g11[i,j-1]
    T['CG3'] = (
        [(I1, G(G00), 1), (D1, G(G00), 0)] + ([(P125, P(C_), 0)] if l else [])
        + [(Neg2I, G(G11), 0)],
        [(U1, G(G11), 0)] + ([(P22, P(B_), 0)] if f else [])
        + [(I1, G(G11), -1), (Neg2I, G(G00), 0)],
    )
    return T


_TAPS = {k: _tap_table(k) for k in ('f', 'm', 'l')}
NW = len(_mats)
WBANDS = np.concatenate([m.astype(BFNP) for m in _mats], axis=1)  # [128, NW*128]


def pv(t, o0, o1, w=768, step=1):
    """Pair view: [128][2 halves at col offsets o0/o1][w cols at `step`]."""
    b = t[:]
    assert o1 > o0, (o0, o1)
    return bass.AP(tensor=b.tensor, offset=b.offset + o0,
                   ap=[[b.ap[0][0], 128], [o1 - o0, 2], [step, w]])


def build_nc():
    nc = bacc.Bacc("TRN2", target_bir_lowering=False, debug=False, num_devices=NCORES)
    x_in = nc.declare_dram_parameter("x", [4, H, H], BF, isOutput=False)
    xr_in = nc.declare_dram_parameter("xr", [4, H, H], BF, isOutput=False)
    wb_in = nc.declare_dram_parameter("wb", [128, NW * 128], BF, isOutput=False)
    out = nc.declare_dram_parameter("out", [3, 2 * H, 2 * H], BF, isOutput=True)
    out_v = out[:].rearrange("c (r two) w -> c r (two w)", two=2)
    x_r = x_in[:].rearrange("c r w -> r c w")
    xr_r = xr_in[:].rearrange("c r w -> r c w")

    with tile.TileContext(nc) as tc, ExitStack() as ctx:
        wpool = ctx.enter_context(tc.tile_pool(name="wpool", bufs=1))
        pl = ctx.enter_context(tc.tile_pool(name="planes", bufs=2))
        gp = ctx.enter_context(tc.tile_pool(name="greens", bufs=2))
        tp = ctx.enter_context(tc.tile_pool(name="tmps", bufs=16))
        qp_ = ctx.enter_context(tc.tile_pool(name="qtmps", bufs=6))
        mp = ctx.enter_context(tc.tile_pool(name="msks", bufs=2))
        op = ctx.enter_context(tc.tile_pool(name="outs", bufs=2))
        pp = ctx.enter_context(tc.tile_pool(name="psums", bufs=1, space="PSUM"))

        WS = wpool.tile([128, NW * 128], BF, tag="ws", name="ws")
        nc.sync.dma_start(WS[:], wb_in[:])

        for si, r0 in enumerate(STRIPS):
            kind = 'f' if si == 0 else ('l' if si == len(STRIPS) - 1 else 'm')
            taps = _TAPS[kind]
            base = r0 - 2

            psum_ctr = [0]
            P2 = pl.tile([128, 6 * PW], BF, tag="P2", name=f"P2_{si}")
            P2r = pl.tile([128, 4 * PW + 2], BF, tag="P2r", name=f"P2r_{si}")
            G2 = gp.tile([128, 2 * PW], BF, tag="G2", name=f"G2_{si}")
            O = op.tile([128, 3 * 3072], BF, tag="O", name=f"O_{si}")

            def tmp(tag):
                return tp.tile([128, 1536], BF, tag="t", name=f"{tag}_{si}")

            def qf(tag):
                return qp_.tile([128, 1536], F, tag="q", name=f"{tag}_{si}")

            def msk(tag):
                return mp.tile([128, 1536], U8, tag=tag, name=f"{tag}_{si}")

            def psum(tag):
                i = psum_ctr[0] % 4
                psum_ctr[0] += 1
                return pp.tile([128, 768], F, tag=f"u{i}", name=f"{tag}_{si}")

            def emit_half(ps, tl):
                for (c0, c1) in ((0, 512), (512, 768)):
                    n = len(tl)
                    for i, (wi, (sk, sb), dlt) in enumerate(tl):
                        src = {'P': P2, 'G': G2, 'R': P2r}[sk]
                        nc.tensor.matmul(
                            ps[:, c0:c1],
                            WS[:, wi * 128:(wi + 1) * 128],
                            src[:, sb + 1 + dlt + c0:sb + 1 + dlt + c1],
                            start=(i == 0), stop=(i == n - 1))

            # ---- input DMA (planes A..D at P2 cols 770..3849) ----
            # compute-engine partition starts must be 32-aligned: memset a whole
            # 32-lane block first, then let the DMA overwrite the loaded lanes.
            if kind == 'f':
                nc.gpsimd.memset(P2[0:32, :], 0.0)
                nc.gpsimd.memset(P2r[0:32, :], 0.0)
            if kind == 'l':
                nc.gpsimd.memset(P2[96:128, :], 0.0)
                nc.gpsimd.memset(P2r[96:128, :], 0.0)
            clo, chi = max(base, 0), min(base + 128, H)
            Pr = P2[:].rearrange("p (six w) -> p six w", six=6)
            nc.sync.dma_start(Pr[clo - base: chi - base, 1:5, 1:1 + H], x_r[clo:chi, :, :])
            _b = P2r[clo - base: chi - base, :]
            Prr = bass.AP(tensor=_b.tensor, offset=_b.offset + RA + 1,
                          ap=[_b.ap[0], [PW, 4], [1, H]])
            nc.sync.dma_start(Prr, xr_r[clo:chi, :, :])

            # ---- plane halo cols ----
            # A.r and D.l are ZERO (masked-channel replication) — the two wide
            # ops that instead need the mosaic replication (c2 pair) get tiny
            # single-column fixups below.
            cc = nc.vector.tensor_copy
            cc(P2[:, A_:A_ + 1], P2[:, A_ + 1:A_ + 2])          # A.l = A[0]
            cc(P2[:, B_:B_ + 1], P2[:, A_ + 1:A_ + 2])          # B.l = A[0]
            nc.vector.memset(P2[:, D_:D_ + 1], 0.0)             # D.l = 0
            nc.vector.memset(P2[:, A_ + 769:A_ + 770], 0.0)     # A.r = 0
            cc(P2[:, C_ + 769:C_ + 770], P2[:, D_ + 768:D_ + 769])  # C.r = D[767]
            cc(P2[:, D_ + 769:D_ + 770], P2[:, D_ + 768:D_ + 769])  # D.r = D[767]
            cc(P2r[:, RA:RA + 1], P2r[:, RA + 1:RA + 2])
            cc(P2r[:, RB:RB + 1], P2r[:, RA + 1:RA + 2])
            nc.vector.memset(P2r[:, RD:RD + 1], 0.0)
            nc.vector.memset(P2r[:, RA + 769:RA + 770], 0.0)
            cc(P2r[:, RC + 769:RC + 770], P2r[:, RD + 768:RD + 769])
            cc(P2r[:, RD + 769:RD + 770], P2r[:, RD + 768:RD + 769])
            # green halo cols (read only P2; emitted early so G-dependent PE
            # matmuls wait only on the green interior writes)
            cc(G2[:, G00 + 769:G00 + 770], P2[:, B_ + 768:B_ + 769])  # g00.r = B[767]
            cc(G2[:, G11:G11 + 1], P2[:, C_ + 1:C_ + 2])              # g11.l = C[0]

            # ---- green: PE verticals (consumers emitted in psum-alloc order
            # so the in-order ACT queue can never cycle with PE buffer reuse) ----
            tV = taps['VD']; tC = taps['C3']; tH = taps['HD']; tW = taps['C2']
            psVD0 = psum("vd0"); emit_half(psVD0, tV[0])
            q5 = qf("q5"); nc.scalar.activation(q5[:, 0:768], psVD0[:], AF.Abs)
            psVD1 = psum("vd1"); emit_half(psVD1, tV[1])
            nc.scalar.activation(q5[:, 768:1536], psVD1[:], AF.Abs)
            psC30 = psum("c30"); emit_half(psC30, tC[0])
            q3 = qf("q3"); nc.scalar.activation(q3[:, 0:768], psC30[:], AF.Abs)
            psC31 = psum("c31"); emit_half(psC31, tC[1])
            nc.scalar.activation(q3[:, 768:1536], psC31[:], AF.Abs)

            # ---- green: horizontals ----
            hs = tmp("hs"); nc.vector.tensor_tensor(hs[:], pv(P2, B_, C_ + 1), pv(P2, B_ + 1, C_ + 2), AL.add)
            psHD0 = psum("hd0"); emit_half(psHD0, tH[0])
            q4 = qf("q4"); nc.scalar.activation(q4[:, 0:768], psHD0[:], AF.Abs)
            psHD1 = psum("hd1"); emit_half(psHD1, tH[1])
            nc.scalar.activation(q4[:, 768:1536], psHD1[:], AF.Abs)
            psC20 = psum("c20"); emit_half(psC20, tW[0])
            # c2 wants mosaic replication at the zeroed A.r/D.l halos (main+residual):
            nc.vector.tensor_tensor(psC20[:, 767:768], psC20[:, 767:768], P2[:, B_ + 768:B_ + 769], AL.add)
            nc.vector.tensor_tensor(psC20[:, 767:768], psC20[:, 767:768], P2r[:, RB + 768:RB + 769], AL.add)
            b2 = tmp("b2"); nc.vector.scalar_tensor_tensor(b2[:, 0:768], psC20[:], -0.5, hs[:, 0:768], AL.mult, AL.add)
            q2 = qf("q2"); nc.scalar.activation(q2[:, 0:768], psC20[:], AF.Abs)
            psC21 = psum("c21"); emit_half(psC21, tW[1])
            nc.vector.tensor_tensor(psC21[:, 0:1], psC21[:, 0:1], P2[:, C_ + 1:C_ + 2], AL.add)
            nc.vector.tensor_tensor(psC21[:, 0:1], psC21[:, 0:1], P2r[:, RC + 1:RC + 2], AL.add)
            nc.vector.scalar_tensor_tensor(b2[:, 768:1536], psC21[:], -0.5, hs[:, 768:1536], AL.mult, AL.add)
            nc.scalar.activation(q2[:, 768:1536], psC21[:], AF.Abs)

            clh = qf("clh"); nc.gpsimd.tensor_tensor(clh[:], q4[:], q2[:], AL.add)
            clv = qf("clv"); nc.gpsimd.tensor_tensor(clv[:], q5[:], q3[:], AL.add)
            dt = qf("dt"); nc.gpsimd.tensor_tensor(dt[:], clh[:], clv[:], AL.subtract)
            mk = msk("mk"); nc.vector.tensor_scalar(mk[:], dt[:], 0.0, None, AL.is_gt)

            tR = taps['RV']
            gint = pv(G2, G00 + 1, G11 + 1)
            nc.vector.tensor_scalar(gint, b2[:], 0.5, None, AL.mult)
            psRV0 = psum("rv0"); emit_half(psRV0, tR[0])
            nc.vector.copy_predicated(G2[:, G00 + 1:G00 + 769], mk[:, 0:768], psRV0[:])
            psRV1 = psum("rv1"); emit_half(psRV1, tR[1])
            nc.vector.copy_predicated(G2[:, G11 + 1:G11 + 769], mk[:, 768:1536], psRV1[:])


            # green outputs: raw (0,1),(1,0) and computed (0,0),(1,1)
            nc.gpsimd.tensor_copy(pv(O, CG + 1, CG + 1536, step=2), pv(P2, B_ + 1, C_ + 1))
            nc.gpsimd.tensor_copy(pv(O, CG, CG + 1537, step=2), gint)

            # ---- chan (R11 | B00) ----
            tS = taps['S']
            psS0 = psum("s0"); emit_half(psS0, tS[0])
            nc.scalar.copy(P2[:, AU + 1:AU + 769], psS0[:])
            nc.vector.memset(P2[:, AU + 769:AU + 770], 0.0)
            psS1 = psum("s1"); emit_half(psS1, tS[1])
            nc.scalar.copy(P2[:, DD + 1:DD + 769], psS1[:])
            nc.vector.memset(P2[:, DD:DD + 1], 0.0)

            rp = tmp("rp"); nc.gpsimd.tensor_tensor(rp[:], pv(P2, A_ + 1, DD), pv(P2, AU + 2, D_ + 1), AL.add)
            rn = tmp("rn"); nc.gpsimd.tensor_tensor(rn[:], pv(P2, A_ + 2, DD + 1), pv(P2, AU + 1, D_), AL.add)

            tDM = taps['DM']; tDN = taps['DN']
            psDM0 = psum("dm0"); emit_half(psDM0, tDM[0])
            qdm = qf("qdm"); nc.scalar.activation(qdm[:, 0:768], psDM0[:], AF.Abs)
            psDM1 = psum("dm1"); emit_half(psDM1, tDM[1])
            nc.scalar.activation(qdm[:, 768:1536], psDM1[:], AF.Abs)
            psDN0 = psum("dn0"); emit_half(psDN0, tDN[0])
            qdn = qf("qdn"); nc.scalar.activation(qdn[:, 0:768], psDN0[:], AF.Abs)
            psDN1 = psum("dn1"); emit_half(psDN1, tDN[1])
            nc.scalar.activation(qdn[:, 768:1536], psDN1[:], AF.Abs)

            # ---- hv fields ----
            tU = taps['U']
            psU0 = psum("u0"); emit_half(psU0, tU[0])
            psU1 = psum("u1"); emit_half(psU1, tU[1])

            t1h = tmp("t1h"); nc.vector.tensor_tensor(t1h[:], pv(P2, A_ + 1, D_), pv(P2, A_ + 2, D_ + 1), AL.add)
            t2h = tmp("t2h"); nc.vector.tensor_tensor(t2h[:], pv(G2, G00 + 1, G11), pv(G2, G00 + 2, G11 + 1), AL.add)
            uh = tmp("uh"); nc.vector.scalar_tensor_tensor(uh[:], t2h[:], -0.5, t1h[:], AL.mult, AL.add)

            # hbc halves are 0.5*B | 0.5*C; the R10/B01 osl ops use them swapped
            hbc = tmp("hbc"); nc.vector.tensor_scalar(hbc[:], pv(P2, B_ + 1, C_ + 1), 0.5, None, AL.mult)

            nc.vector.scalar_tensor_tensor(pv(O, CR + 1, CB + 1536, step=2), uh[:], 0.5, hbc[:], AL.mult, AL.add)
            _d = pv(O, CR + 1536, CB + 1, step=2)
            _d0 = bass.AP(tensor=_d.tensor, offset=_d.offset, ap=[_d.ap[0], [2, 768]])
            _d1 = bass.AP(tensor=_d.tensor, offset=_d.offset + (CB + 1 - (CR + 1536)), ap=[_d.ap[0], [2, 768]])
            nc.vector.scalar_tensor_tensor(_d0, psU0[:], 0.5, hbc[:, 768:1536], AL.mult, AL.add)
            nc.vector.scalar_tensor_tensor(_d1, psU1[:], 0.5, hbc[:, 0:768], AL.mult, AL.add)

            tG2_ = taps['CG2']; tG3_ = taps['CG3']
            psCG20 = psum("g20"); emit_half(psCG20, tG2_[0])
            qp = qf("qp"); nc.scalar.activation(qp[:, 0:768], psCG20[:], AF.Abs)
            cp2 = tmp("cp2"); nc.vector.scalar_tensor_tensor(cp2[:, 0:768], psCG20[:], -0.5, rp[:, 0:768], AL.mult, AL.add)
            psCG21 = psum("g21"); emit_half(psCG21, tG2_[1])
            nc.scalar.activation(qp[:, 768:1536], psCG21[:], AF.Abs)
            nc.vector.scalar_tensor_tensor(cp2[:, 768:1536], psCG21[:], -0.5, rp[:, 768:1536], AL.mult, AL.add)
            psCG30 = psum("g30"); emit_half(psCG30, tG3_[0])
            qn = qf("qn"); nc.scalar.activation(qn[:, 0:768], psCG30[:], AF.Abs)
            cn2 = tmp("cn2"); nc.vector.scalar_tensor_tensor(cn2[:, 0:768], psCG30[:], -0.5, rn[:, 0:768], AL.mult, AL.add)
            psCG31 = psum("g31"); emit_half(psCG31, tG3_[1])
            nc.scalar.activation(qn[:, 768:1536], psCG31[:], AF.Abs)
            nc.vector.scalar_tensor_tensor(cn2[:, 768:1536], psCG31[:], -0.5, rn[:, 768:1536], AL.mult, AL.add)
            clp = qf("clp"); nc.gpsimd.tensor_tensor(clp[:], qdm[:], qp[:], AL.add)
            cln = qf("cln"); nc.gpsimd.tensor_tensor(cln[:], qdn[:], qn[:], AL.add)
            dr = qf("dr"); nc.gpsimd.tensor_tensor(dr[:], clp[:], cln[:], AL.subtract)
            mr = msk("mr"); nc.vector.tensor_scalar(mr[:], dr[:], 0.0, None, AL.is_gt)
            cnh = tmp("cnh"); nc.vector.tensor_scalar(cnh[:], cn2[:], 0.5, None, AL.mult)
            odst = pv(O, CR + 1537, CB, step=2)
            nc.scalar.mul(odst, cp2[:], 0.5)
            nc.vector.copy_predicated(odst, mr[:], cnh[:])

            # ---- raw red/blue phases ----
            nc.gpsimd.tensor_copy(pv(O, CR, CB + 1537, step=2), pv(P2, A_ + 1, D_ + 1))

            # ---- output DMA (3 colors on 3 queues) ----
            if kind == 'l':
                p0, pn, row0 = 102, 24, 744
            else:
                p0, pn, row0 = 2, 124, r0
            nc.sync.dma_start(out_v[0, row0:row0 + pn, :], O[p0:p0 + pn, CR:CR + 3072])
            nc.scalar.dma_start(out_v[1, row0:row0 + pn, :], O[p0:p0 + pn, CG:CG + 3072])
            nc.gpsimd.dma_start(out_v[2, row0:row0 + pn, :], O[p0:p0 + pn, CB:CB + 3072])

    nc.compile()
    return nc


_NC_CACHE = None


def kernel(x: np.ndarray) -> np.ndarray:
    global _NC_CACHE
    if _NC_CACHE is None:
        _NC_CACHE = build_nc()
    xb = np.ascontiguousarray(x.astype(BFNP))
    xr = np.ascontiguousarray((x.astype(np.float32) - xb.astype(np.float32)).astype(BFNP))
    wb = np.ascontiguousarray(WBANDS)
    in_maps = [{"x": xb[i], "xr": xr[i], "wb": wb} for i in range(NCORES)]
    res = run_bass_kernel_spmd(_NC_CACHE, in_maps, list(range(NCORES)))
    return np.stack([res.results[i]["out"].astype(np.float32) for i in range(NCORES)], axis=0)


# revision 9
# speedup vs baseline: 1.1888x; 1.0192x over previous
"""Hamilton-Adams demosaic for Trainium2 — v2 (bf16, PE banded verticals, paired ops).

Input:  x [8, 4, 768, 768] f32  (quarter-res planes A=R, B=Gr, C=Gb, D=B)
Output: [8, 3, 1536, 1536] f32

All device compute in bf16 (host casts in/out).  Every gradient feeding a
directional-selection compare (vdif, c3, hdif, c2, dm, dn) runs on the Tensor
engine as a banded-stationary matmul accumulating BOTH the bf16 input and a
bf16 residual channel (xr = x - bf16(x)) in fp32 PSUM, so selection decisions
are effectively fp32-exact; only output values carry bf16 noise (l2 ~8e-3,
budget 2e-2).  Cross-partition (vertical) stencils and shifts are all PE
matmuls (W [128,128] per band, edge strips get modified bands + patch taps);
horizontal value ops run on DVE; |x| on Act; compares/copies on Pool.  The two
symmetric phases of each computation are fused into single wide ops via 3-dim
access patterns with a per-half column offset; PSUM results use 2-bank
single-half tiles on a 4-tag rotation for pipeline depth.

Tile layouts (halo col each side of every 770-wide plane region):
  P2  [128, 6*770] bf16: [au | A | B | C | D | dd]  (au=A row+1, dd=D row-1)
  P2r [128, 4*770+2] bf16: residual planes [A | B | C | D]
  G2  [128, 2*770] bf16: [g00 | g11]
  O   [128, 3*3072] bf16: interleaved full-res output rows (R | G | B)
"""
import sys
sys.path.insert(0, '/opt/trn_rl_repo')

from contextlib import ExitStack

import numpy as np
import ml_dtypes

import concourse.bass as bass
import concourse.bacc as bacc
import concourse.tile as tile
from concourse import mybir
from concourse.bass_utils import run_bass_kernel_spmd

F = mybir.dt.float32
BF = mybir.dt.bfloat16
U8 = mybir.dt.uint8
AL = mybir.AluOpType
AF = mybir.ActivationFunctionType
BFNP = ml_dtypes.bfloat16

H = 768
PW = 770
NCORES = 8
STRIPS = [0, 124, 248, 372, 496, 620, 644]

# P2 region base cols
AU, A_, B_, C_, D_, DD = 0, 770, 1540, 2310, 3080, 3850
# P2r (bf16 residual planes) region base cols — offset by 1 so taps with
# column delta -1/+1 on the first/last region stay inside the tile
RA, RB, RC, RD = 1, 771, 1541, 2311
# G2 region base cols
G00, G11 = 0, 770
# O color base cols
CR, CG, CB = 0, 3072, 6144

# ---------------- host-side band matrices ----------------
_mats: list[np.ndarray] = []
_mat_idx: dict = {}


def _W(taps=None, colfix=None):
    taps = taps or {}
    colfix = colfix or {}
    key = (tuple(sorted(taps.items())),
           tuple(sorted((m, tuple(sorted(f.items()))) for m, f in colfix.items())))
    if key in _mat_idx:
        return _mat_idx[key]
    M = np.zeros((128, 128), np.float32)
    for m in range(128):
        if m in colfix:
            for k, w in colfix[m].items():
                M[k, m] = w
        else:
            for d, w in taps.items():
                k = m + d
                if 0 <= k < 128:
                    M[k, m] = w
    idx = len(_mats)
    _mats.append(M)
    _mat_idx[key] = idx
    return idx


def _tap_table(kind):
    """kind in 'f' (first strip), 'm' (mid), 'l' (last).
    Returns result -> (half0 taps, half1 taps); tap = (Widx, src_region).
    src_region is a P2 or G2 base col; G-sourced taps marked by ('G', base)."""
    f, l = kind == 'f', kind == 'l'

    def FF(fix):
        return {0: {}, 1: {}, **fix}

    def LL(fix):
        return {**fix, 126: {}, 127: {}}

    def BW(taps, ffix=None, lfix=None):
        cf = None
        if f and ffix is not None:
            cf = FF(ffix)
        if l and lfix is not None:
            cf = LL(lfix)
        return _W(taps, cf)

    P22 = _W({}, {2: {2: 1.0}})
    P125 = _W({}, {125: {125: 1.0}})
    P125n = _W({}, {125: {125: -1.0}})
    P = lambda b: ('P', b)
    G = lambda b: ('G', b)
    R = lambda b: ('R', b)
    RES = {A_: RA, B_: RB, C_: RC, D_: RD}

    def with_res(taps):
        # duplicate every P-plane tap onto the residual tile (same W, same delta)
        out = list(taps)
        for wi, (sk, sb), dlt in taps:
            if sk == 'P':
                out.append((wi, R(RES[sb]), dlt))
        return out

    I1 = _W({0: 1.0})
    NegI = _W({0: -1.0})
    Neg2I = _W({0: -2.0})
    D1 = _W({1: 1.0}, LL({125: {}}) if l else None)   # down-shift; zero bottom edge
    U1 = _W({-1: 1.0}, FF({2: {}}) if f else None)    # up-shift; zero top edge
    NegU1 = _W({-1: -1.0}, FF({2: {}}) if f else None)

    T = {}
    # VD: vdif00 = C[i-1]-C[i]  |  vdif11 = B[i]-B[i+1]   (exact: + residual taps)
    T['VD'] = (
        with_res([(BW({-1: 1, 0: -1}, ffix={2: {2: -1}}), P(C_), 0)] + ([(P22, P(A_), 0)] if f else [])),
        with_res([(BW({0: 1, 1: -1}, lfix={125: {125: 1}}), P(B_), 0)] + ([(P125n, P(D_), 0)] if l else [])),
    )
    # C3: A[i-1]-2A[i]+A[i+1]  |  D[i-1]-2D[i]+D[i+1]
    T['C3'] = (
        with_res([(BW({-1: 1, 0: -2, 1: 1}, ffix={2: {2: -1, 3: 1}}, lfix={125: {124: 1, 125: -2}}), P(A_), 0)]
                 + ([(P125, P(C_), 0)] if l else [])),
        with_res([(BW({-1: 1, 0: -2, 1: 1}, ffix={2: {2: -2, 3: 1}}, lfix={125: {124: 1, 125: -1}}), P(D_), 0)]
                 + ([(P22, P(B_), 0)] if f else [])),
    )
    # HD: hdif00 = B[j-1]-B[j]  |  hdif11 = C[j]-C[j+1]   (horizontal, via col deltas)
    T['HD'] = (
        with_res([(I1, P(B_), -1), (NegI, P(B_), 0)]),
        with_res([(I1, P(C_), 0), (NegI, P(C_), 1)]),
    )
    # C2: A[j-1]-2A[j]+A[j+1]  |  D[j-1]-2D[j]+D[j+1]  (edge cols fixed on psum)
    T['C2'] = (
        with_res([(I1, P(A_), -1), (Neg2I, P(A_), 0), (I1, P(A_), 1)]),
        with_res([(I1, P(D_), -1), (Neg2I, P(D_), 0), (I1, P(D_), 1)]),
    )
    # RV: rawv = 0.5*vsum - 0.25*c3 (the vertical green candidate, final scale)
    T['RV'] = (
        [(BW({-1: .5, 0: .5}, ffix={2: {2: .5}}, lfix={125: {124: .5, 125: .25}}), P(C_), 0),
         (BW({-1: -.25, 0: .5, 1: -.25}, ffix={2: {2: .75, 3: -.25}},
             lfix={125: {124: -.25, 125: .5}}), P(A_), 0)],
        [(BW({0: .5, 1: .5}, ffix={2: {2: .25, 3: .5}}, lfix={125: {125: .5}}), P(B_), 0),
         (BW({-1: -.25, 0: .5, 1: -.25}, ffix={2: {2: .5, 3: -.25}},
             lfix={125: {124: -.25, 125: .75}}), P(D_), 0)],
    )
    # U = t1 - 0.5*t2 for the vertical hv pair (R10 | B01):
    #   R10: (A[i]+A[i+1]) - 0.5*(g00[i]+g00[i+1])   (bottom edge: A only / +C patch)
    #   B01: (D[i-1]+D[i]) - 0.5*(g11[i-1]+g11[i])   (top edge: D only / +B patch)
    P125h = _W({}, {125: {125: -0.5}})
    P22h = _W({}, {2: {2: -0.5}})
    T['U'] = (
        [(BW({0: 1, 1: 1}, lfix={125: {125: 1}}), P(A_), 0),
         (BW({0: -.5, 1: -.5}, lfix={125: {125: -.5}}), G(G00), 0)]
        + ([(P125h, P(C_), 0)] if l else []),
        [(BW({-1: 1, 0: 1}, ffix={2: {2: 1}}), P(D_), 0),
         (BW({-1: -.5, 0: -.5}, ffix={2: {2: -.5}}), G(G11), 0)]
        + ([(P22h, P(B_), 0)] if f else []),
    )
    # S: au = A[i+1] (last: 0)  |  dd = D[i-1] (first: 0)
    T['S'] = (
        [(D1, P(A_), 0)],
        [(U1, P(D_), 0)],
    )
    # DM: |R11| au[j+1]-A[j] = A[i+1,j+1]-A[i,j]  |  |B00| D[i,j]-D[i-1,j-1]
    T['DM'] = (
        [(D1, P(A_), 1), (NegI, P(A_), 0)],
        [(I1, P(D_), 0), (NegU1, P(D_), -1)],
    )
    # DN: R11: A[i+1,j]-A[i,j+1]  |  B00: D[i,j-1]-D[i-1,j]
    T['DN'] = (
        [(D1, P(A_), 0), (NegI, P(A_), 1)],
        [(I1, P(D_), -1), (NegU1, P(D_), 0)],
    )
    # CG2 = sp - 2*gnear;  sp_R11 = g00[i,j]+g00[i+1,j+1], sp_B00 = g11[i-1,j-1]+g11[i,j]
    T['CG2'] = (
        [(I1, G(G00), 0), (D1, G(G00), 1)] + ([(P125, P(C_), 1)] if l else [])
        + [(Neg2I, G(G11), 0)],
        [(U1, G(G11), -1)] + ([(P22, P(B_), -1)] if f else [])
        + [(I1, G(G11), 0), (Neg2I, G(G00), 0)],
    )
    # CG3 = sn - 2*gnear;  sn_R11 = g00[i,j+1]+g00[i+1,j], sn_B00 = g11[i-1,j]+g11[i,j-1]
    T['CG3'] = (
        [(I1, G(G00), 1), (D1, G(G00), 0)] + ([(P125, P(C_), 0)] if l else [])
        + [(Neg2I, G(G11), 0)],
        [(U1, G(G11), 0)] + ([(P22, P(B_), 0)] if f else [])
        + [(I1, G(G11), -1), (Neg2I, G(G00), 0)],
    )
    return T


_TAPS = {k: _tap_table(k) for k in ('f', 'm', 'l')}
NW = len(_mats)
WBANDS = np.concatenate([m.astype(BFNP) for m in _mats], axis=1)  # [128, NW*128]


def pv(t, o0, o1, w=768, step=1):
    """Pair view: [128][2 halves at col offsets o0/o1][w cols at `step`]."""
    b = t[:]
    assert o1 > o0, (o0, o1)
    return bass.AP(tensor=b.tensor, offset=b.offset + o0,
                   ap=[[b.ap[0][0], 128], [o1 - o0, 2], [step, w]])


def build_nc():
    nc = bacc.Bacc("TRN2", target_bir_lowering=False, debug=False, num_devices=NCORES)
    x_in = nc.declare_dram_parameter("x", [4, H, H], BF, isOutput=False)
    xr_in = nc.declare_dram_parameter("xr", [4, H, H], BF, isOutput=False)
    wb_in = nc.declare_dram_parameter("wb", [128, NW * 128], BF, isOutput=False)
    out = nc.declare_dram_parameter("out", [3, 2 * H, 2 * H], BF, isOutput=True)
    out_v = out[:].rearrange("c (r two) w -> c r (two w)", two=2)
    x_r = x_in[:].rearrange("c r w -> r c w")
    xr_r = xr_in[:].rearrange("c r w -> r c w")

    with tile.TileContext(nc) as tc, ExitStack() as ctx:
        wpool = ctx.enter_context(tc.tile_pool(name="wpool", bufs=1))
        pl = ctx.enter_context(tc.tile_pool(name="planes", bufs=2))
        gp = ctx.enter_context(tc.tile_pool(name="greens", bufs=2))
        tp = ctx.enter_context(tc.tile_pool(name="tmps", bufs=16))
        qp_ = ctx.enter_context(tc.tile_pool(name="qtmps", bufs=6))
        mp = ctx.enter_context(tc.tile_pool(name="msks", bufs=2))
        op = ctx.enter_context(tc.tile_pool(name="outs", bufs=2))
        pp = ctx.enter_context(tc.tile_pool(name="psums", bufs=1, space="PSUM"))

        WS = wpool.tile([128, NW * 128], BF, tag="ws", name="ws")
        nc.sync.dma_start(WS[:], wb_in[:])

        pending_out = [None]

        def flush_out():
            if pending_out[0] is None:
                return
            Op, p0p, pnp, r0p = pending_out[0]
            nc.sync.dma_start(out_v[0, r0p:r0p + pnp, :], Op[p0p:p0p + pnp, CR:CR + 3072])
            nc.scalar.dma_start(out_v[1, r0p:r0p + pnp, :], Op[p0p:p0p + pnp, CG:CG + 3072])
            nc.sync.dma_start(out_v[2, r0p:r0p + pnp, :], Op[p0p:p0p + pnp, CB:CB + 3072])
            pending_out[0] = None

        for si, r0 in enumerate(STRIPS):
            kind = 'f' if si == 0 else ('l' if si == len(STRIPS) - 1 else 'm')
            taps = _TAPS[kind]
            base = r0 - 2

            psum_ctr = [0]
            P2 = pl.tile([128, 6 * PW], BF, tag="P2", name=f"P2_{si}")
            P2r = pl.tile([128, 4 * PW + 2], BF, tag="P2r", name=f"P2r_{si}")
            G2 = gp.tile([128, 2 * PW], BF, tag="G2", name=f"G2_{si}")
            O = op.tile([128, 3 * 3072], BF, tag="O", name=f"O_{si}")

            def tmp(tag):
                return tp.tile([128, 1536], BF, tag="t", name=f"{tag}_{si}")

            def qf(tag):
                return qp_.tile([128, 1536], F, tag="q", name=f"{tag}_{si}")

            def msk(tag):
                return mp.tile([128, 1536], U8, tag=tag, name=f"{tag}_{si}")

            def psum(tag):
                i = psum_ctr[0] % 4
                psum_ctr[0] += 1
                return pp.tile([128, 768], F, tag=f"u{i}", name=f"{tag}_{si}")

            def emit_half(ps, tl):
                for (c0, c1) in ((0, 512), (512, 768)):
                    n = len(tl)
                    for i, (wi, (sk, sb), dlt) in enumerate(tl):
                        src = {'P': P2, 'G': G2, 'R': P2r}[sk]
                        nc.tensor.matmul(
                            ps[:, c0:c1],
                            WS[:, wi * 128:(wi + 1) * 128],
                            src[:, sb + 1 + dlt + c0:sb + 1 + dlt + c1],
                            start=(i == 0), stop=(i == n - 1))

            # ---- input DMA (planes A..D at P2 cols 770..3849) ----
            # compute-engine partition starts must be 32-aligned: memset a whole
            # 32-lane block first, then let the DMA overwrite the loaded lanes.
            if kind == 'f':
                nc.gpsimd.memset(P2[0:32, :], 0.0)
                nc.gpsimd.memset(P2r[0:32, :], 0.0)
            if kind == 'l':
                nc.gpsimd.memset(P2[96:128, :], 0.0)
                nc.gpsimd.memset(P2r[96:128, :], 0.0)
            clo, chi = max(base, 0), min(base + 128, H)
            Pr = P2[:].rearrange("p (six w) -> p six w", six=6)
            nc.sync.dma_start(Pr[clo - base: chi - base, 1:5, 1:1 + H], x_r[clo:chi, :, :])
            _b = P2r[clo - base: chi - base, :]
            Prr = bass.AP(tensor=_b.tensor, offset=_b.offset + RA + 1,
                          ap=[_b.ap[0], [PW, 4], [1, H]])
            nc.sync.dma_start(Prr, xr_r[clo:chi, :, :])
            flush_out()

            # ---- plane halo cols ----
            # A.r and D.l are ZERO (masked-channel replication) — the two wide
            # ops that instead need the mosaic replication (c2 pair) get tiny
            # single-column fixups below.
            cc = nc.vector.tensor_copy
            cc(P2[:, A_:A_ + 1], P2[:, A_ + 1:A_ + 2])          # A.l = A[0]
            cc(P2[:, B_:B_ + 1], P2[:, A_ + 1:A_ + 2])          # B.l = A[0]
            nc.vector.memset(P2[:, D_:D_ + 1], 0.0)             # D.l = 0
            nc.vector.memset(P2[:, A_ + 769:A_ + 770], 0.0)     # A.r = 0
            cc(P2[:, C_ + 769:C_ + 770], P2[:, D_ + 768:D_ + 769])  # C.r = D[767]
            cc(P2[:, D_ + 769:D_ + 770], P2[:, D_ + 768:D_ + 769])  # D.r = D[767]
            cc(P2r[:, RA:RA + 1], P2r[:, RA + 1:RA + 2])
            cc(P2r[:, RB:RB + 1], P2r[:, RA + 1:RA + 2])
            nc.vector.memset(P2r[:, RD:RD + 1], 0.0)
            nc.vector.memset(P2r[:, RA + 769:RA + 770], 0.0)
            cc(P2r[:, RC + 769:RC + 770], P2r[:, RD + 768:RD + 769])
            cc(P2r[:, RD + 769:RD + 770], P2r[:, RD + 768:RD + 769])
            # green halo cols (read only P2; emitted early so G-dependent PE
            # matmuls wait only on the green interior writes)
            cc(G2[:, G00 + 769:G00 + 770], P2[:, B_ + 768:B_ + 769])  # g00.r = B[767]
            cc(G2[:, G11:G11 + 1], P2[:, C_ + 1:C_ + 2])              # g11.l = C[0]

            # ---- green: PE verticals (consumers emitted in psum-alloc order
            # so the in-order ACT queue can never cycle with PE buffer reuse) ----
            tV = taps['VD']; tC = taps['C3']; tH = taps['HD']; tW = taps['C2']
            psVD0 = psum("vd0"); emit_half(psVD0, tV[0])
            q5 = qf("q5"); nc.scalar.activation(q5[:, 0:768], psVD0[:], AF.Abs)
            psVD1 = psum("vd1"); emit_half(psVD1, tV[1])
            nc.scalar.activation(q5[:, 768:1536], psVD1[:], AF.Abs)
            psC30 = psum("c30"); emit_half(psC30, tC[0])
            q3 = qf("q3"); nc.scalar.activation(q3[:, 0:768], psC30[:], AF.Abs)
            psC31 = psum("c31"); emit_half(psC31, tC[1])
            nc.scalar.activation(q3[:, 768:1536], psC31[:], AF.Abs)

            # ---- green: horizontals ----
            hs = tmp("hs"); nc.vector.tensor_tensor(hs[:], pv(P2, B_, C_ + 1), pv(P2, B_ + 1, C_ + 2), AL.add)
            psHD0 = psum("hd0"); emit_half(psHD0, tH[0])
            q4 = qf("q4"); nc.scalar.activation(q4[:, 0:768], psHD0[:], AF.Abs)
            psHD1 = psum("hd1"); emit_half(psHD1, tH[1])
            nc.scalar.activation(q4[:, 768:1536], psHD1[:], AF.Abs)
            psC20 = psum("c20"); emit_half(psC20, tW[0])
            # c2 wants mosaic replication at the zeroed A.r/D.l halos (main+residual):
            nc.vector.tensor_tensor(psC20[:, 767:768], psC20[:, 767:768], P2[:, B_ + 768:B_ + 769], AL.add)
            nc.vector.tensor_tensor(psC20[:, 767:768], psC20[:, 767:768], P2r[:, RB + 768:RB + 769], AL.add)
            b2 = tmp("b2"); nc.vector.scalar_tensor_tensor(b2[:, 0:768], psC20[:], -0.5, hs[:, 0:768], AL.mult, AL.add)
            q2 = qf("q2"); nc.scalar.activation(q2[:, 0:768], psC20[:], AF.Abs)
            psC21 = psum("c21"); emit_half(psC21, tW[1])
            nc.vector.tensor_tensor(psC21[:, 0:1], psC21[:, 0:1], P2[:, C_ + 1:C_ + 2], AL.add)
            nc.vector.tensor_tensor(psC21[:, 0:1], psC21[:, 0:1], P2r[:, RC + 1:RC + 2], AL.add)
            nc.vector.scalar_tensor_tensor(b2[:, 768:1536], psC21[:], -0.5, hs[:, 768:1536], AL.mult, AL.add)
            nc.scalar.activation(q2[:, 768:1536], psC21[:], AF.Abs)

            clh = qf("clh"); nc.gpsimd.tensor_tensor(clh[:], q4[:], q2[:], AL.add)
            clv = qf("clv"); nc.gpsimd.tensor_tensor(clv[:], q5[:], q3[:], AL.add)
            mk = msk("mk"); nc.vector.tensor_tensor(mk[:], clh[:], clv[:], AL.is_gt)

            tR = taps['RV']
            gint = pv(G2, G00 + 1, G11 + 1)
            nc.vector.tensor_scalar(gint, b2[:], 0.5, None, AL.mult)
            psRV0 = psum("rv0"); emit_half(psRV0, tR[0])
            nc.vector.copy_predicated(G2[:, G00 + 1:G00 + 769], mk[:, 0:768], psRV0[:])
            psRV1 = psum("rv1"); emit_half(psRV1, tR[1])
            nc.vector.copy_predicated(G2[:, G11 + 1:G11 + 769], mk[:, 768:1536], psRV1[:])


            # green outputs: raw (0,1),(1,0) and computed (0,0),(1,1)
            nc.gpsimd.tensor_copy(pv(O, CG + 1, CG + 1536, step=2), pv(P2, B_ + 1, C_ + 1))
            nc.gpsimd.tensor_copy(pv(O, CG, CG + 1537, step=2), gint)

            # ---- chan (R11 | B00) ----
            tS = taps['S']
            psS0 = psum("s0"); emit_half(psS0, tS[0])
            nc.scalar.copy(P2[:, AU + 1:AU + 769], psS0[:])
            nc.vector.memset(P2[:, AU + 769:AU + 770], 0.0)
            psS1 = psum("s1"); emit_half(psS1, tS[1])
            nc.scalar.copy(P2[:, DD + 1:DD + 769], psS1[:])
            nc.vector.memset(P2[:, DD:DD + 1], 0.0)

            rp = tmp("rp"); nc.gpsimd.tensor_tensor(rp[:], pv(P2, A_ + 1, DD), pv(P2, AU + 2, D_ + 1), AL.add)
            rn = tmp("rn"); nc.gpsimd.tensor_tensor(rn[:], pv(P2, A_ + 2, DD + 1), pv(P2, AU + 1, D_), AL.add)

            tDM = taps['DM']; tDN = taps['DN']
            psDM0 = psum("dm0"); emit_half(psDM0, tDM[0])
            qdm = qf("qdm"); nc.scalar.activation(qdm[:, 0:768], psDM0[:], AF.Abs)
            psDM1 = psum("dm1"); emit_half(psDM1, tDM[1])
            nc.scalar.activation(qdm[:, 768:1536], psDM1[:], AF.Abs)
            psDN0 = psum("dn0"); emit_half(psDN0, tDN[0])
            qdn = qf("qdn"); nc.scalar.activation(qdn[:, 0:768], psDN0[:], AF.Abs)
            psDN1 = psum("dn1"); emit_half(psDN1, tDN[1])
            nc.scalar.activation(qdn[:, 768:1536], psDN1[:], AF.Abs)

            # ---- hv fields ----
            tU = taps['U']
            psU0 = psum("u0"); emit_half(psU0, tU[0])
            psU1 = psum("u1"); emit_half(psU1, tU[1])

            t1h = tmp("t1h"); nc.vector.tensor_tensor(t1h[:], pv(P2, A_ + 1, D_), pv(P2, A_ + 2, D_ + 1), AL.add)
            t2h = tmp("t2h"); nc.vector.tensor_tensor(t2h[:], pv(G2, G00 + 1, G11), pv(G2, G00 + 2, G11 + 1), AL.add)
            uh = tmp("uh"); nc.vector.scalar_tensor_tensor(uh[:], t2h[:], -0.5, t1h[:], AL.mult, AL.add)

            # hbc halves are 0.5*B | 0.5*C; the R10/B01 osl ops use them swapped
            hbc = tmp("hbc"); nc.vector.tensor_scalar(hbc[:], pv(P2, B_ + 1, C_ + 1), 0.5, None, AL.mult)

            nc.vector.scalar_tensor_tensor(pv(O, CR + 1, CB + 1536, step=2), uh[:], 0.5, hbc[:], AL.mult, AL.add)
            _d = pv(O, CR + 1536, CB + 1, step=2)
            _d0 = bass.AP(tensor=_d.tensor, offset=_d.offset, ap=[_d.ap[0], [2, 768]])
            _d1 = bass.AP(tensor=_d.tensor, offset=_d.offset + (CB + 1 - (CR + 1536)), ap=[_d.ap[0], [2, 768]])
            nc.vector.scalar_tensor_tensor(_d0, psU0[:], 0.5, hbc[:, 768:1536], AL.mult, AL.add)
            nc.vector.scalar_tensor_tensor(_d1, psU1[:], 0.5, hbc[:, 0:768], AL.mult, AL.add)

            tG2_ = taps['CG2']; tG3_ = taps['CG3']
            psCG20 = psum("g20"); emit_half(psCG20, tG2_[0])
            qp = qf("qp"); nc.scalar.activation(qp[:, 0:768], psCG20[:], AF.Abs)
            cp2 = tmp("cp2"); nc.vector.scalar_tensor_tensor(cp2[:, 0:768], psCG20[:], -0.5, rp[:, 0:768], AL.mult, AL.add)
            psCG21 = psum("g21"); emit_half(psCG21, tG2_[1])
            nc.scalar.activation(qp[:, 768:1536], psCG21[:], AF.Abs)
            nc.vector.scalar_tensor_tensor(cp2[:, 768:1536], psCG21[:], -0.5, rp[:, 768:1536], AL.mult, AL.add)
            psCG30 = psum("g30"); emit_half(psCG30, tG3_[0])
            qn = qf("qn"); nc.scalar.activation(qn[:, 0:768], psCG30[:], AF.Abs)
            cn2 = tmp("cn2"); nc.vector.scalar_tensor_tensor(cn2[:, 0:768], psCG30[:], -0.5, rn[:, 0:768], AL.mult, AL.add)
            psCG31 = psum("g31"); emit_half(psCG31, tG3_[1])
            nc.scalar.activation(qn[:, 768:1536], psCG31[:], AF.Abs)
            nc.vector.scalar_tensor_tensor(cn2[:, 768:1536], psCG31[:], -0.5, rn[:, 768:1536], AL.mult, AL.add)
            clp = qf("clp"); nc.gpsimd.tensor_tensor(clp[:], qdm[:], qp[:], AL.add)
            cln = qf("cln"); nc.gpsimd.tensor_tensor(cln[:], qdn[:], qn[:], AL.add)
            mr = msk("mr"); nc.vector.tensor_tensor(mr[:], clp[:], cln[:], AL.is_gt)
            cnh = tmp("cnh"); nc.vector.tensor_scalar(cnh[:], cn2[:], 0.5, None, AL.mult)
            odst = pv(O, CR + 1537, CB, step=2)
            nc.scalar.mul(odst, cp2[:], 0.5)
            nc.vector.copy_predicated(odst, mr[:], cnh[:])

            # ---- raw red/blue phases ----
            nc.gpsimd.tensor_copy(pv(O, CR, CB + 1537, step=2), pv(P2, A_ + 1, D_ + 1))

            # ---- output DMA (3 colors on 3 queues), deferred so the NEXT
            # strip's input DMAs precede them on the SP queue ----
            if kind == 'l':
                p0, pn, row0 = 102, 24, 744
            else:
                p0, pn, row0 = 2, 124, r0
            pending_out[0] = (O, p0, pn, row0)
        flush_out()

    nc.compile()
    return nc


_NC_CACHE = None


def kernel(x: np.ndarray) -> np.ndarray:
    global _NC_CACHE
    if _NC_CACHE is None:
        _NC_CACHE = build_nc()
    xb = np.ascontiguousarray(x.astype(BFNP))
    xr = np.ascontiguousarray((x.astype(np.float32) - xb.astype(np.float32)).astype(BFNP))
    wb = np.ascontiguousarray(WBANDS)
    in_maps = [{"x": xb[i], "xr": xr[i], "wb": wb} for i in range(NCORES)]
    res = run_bass_kernel_spmd(_NC_CACHE, in_maps, list(range(NCORES)))
    return np.stack([res.results[i]["out"].astype(np.float32) for i in range(NCORES)], axis=0)


# revision 10
# speedup vs baseline: 1.1899x; 1.0010x over previous
"""Hamilton-Adams demosaic for Trainium2 — v2 (bf16, PE banded verticals, paired ops).

Input:  x [8, 4, 768, 768] f32  (quarter-res planes A=R, B=Gr, C=Gb, D=B)
Output: [8, 3, 1536, 1536] f32

All device compute in bf16 (host casts in/out).  Every gradient feeding a
directional-selection compare (vdif, c3, hdif, c2, dm, dn) runs on the Tensor
engine as a banded-stationary matmul accumulating BOTH the bf16 input and a
bf16 residual channel (xr = x - bf16(x)) in fp32 PSUM, so selection decisions
are effectively fp32-exact; only output values carry bf16 noise (l2 ~8e-3,
budget 2e-2).  Cross-partition (vertical) stencils and shifts are all PE
matmuls (W [128,128] per band, edge strips get modified bands + patch taps);
horizontal value ops run on DVE; |x| on Act; compares/copies on Pool.  The two
symmetric phases of each computation are fused into single wide ops via 3-dim
access patterns with a per-half column offset; PSUM results use 2-bank
single-half tiles on a 4-tag rotation for pipeline depth.

Tile layouts (halo col each side of every 770-wide plane region):
  P2  [128, 6*770] bf16: [au | A | B | C | D | dd]  (au=A row+1, dd=D row-1)
  P2r [128, 4*770+2] bf16: residual planes [A | B | C | D]
  G2  [128, 2*770] bf16: [g00 | g11]
  O   [128, 3*3072] bf16: interleaved full-res output rows (R | G | B)
"""
import sys
sys.path.insert(0, '/opt/trn_rl_repo')

from contextlib import ExitStack

import numpy as np
import ml_dtypes

import concourse.bass as bass
import concourse.bacc as bacc
import concourse.tile as tile
from concourse import mybir
from concourse.bass_utils import run_bass_kernel_spmd

F = mybir.dt.float32
BF = mybir.dt.bfloat16
U8 = mybir.dt.uint8
AL = mybir.AluOpType
AF = mybir.ActivationFunctionType
BFNP = ml_dtypes.bfloat16

H = 768
PW = 770
NCORES = 8
STRIPS = [0, 124, 248, 372, 496, 620, 644]

# P2 region base cols
AU, A_, B_, C_, D_, DD = 0, 770, 1540, 2310, 3080, 3850
# P2r (bf16 residual planes) region base cols — offset by 1 so taps with
# column delta -1/+1 on the first/last region stay inside the tile
RA, RB, RC, RD = 1, 771, 1541, 2311
# G2 region base cols
G00, G11 = 0, 770
# O color base cols
CR, CG, CB = 0, 3072, 6144

# ---------------- host-side band matrices ----------------
_mats: list[np.ndarray] = []
_mat_idx: dict = {}


def _W(taps=None, colfix=None):
    taps = taps or {}
    colfix = colfix or {}
    key = (tuple(sorted(taps.items())),
           tuple(sorted((m, tuple(sorted(f.items()))) for m, f in colfix.items())))
    if key in _mat_idx:
        return _mat_idx[key]
    M = np.zeros((128, 128), np.float32)
    for m in range(128):
        if m in colfix:
            for k, w in colfix[m].items():
                M[k, m] = w
        else:
            for d, w in taps.items():
                k = m + d
                if 0 <= k < 128:
                    M[k, m] = w
    idx = len(_mats)
    _mats.append(M)
    _mat_idx[key] = idx
    return idx


def _tap_table(kind):
    """kind in 'f' (first strip), 'm' (mid), 'l' (last).
    Returns result -> (half0 taps, half1 taps); tap = (Widx, src_region).
    src_region is a P2 or G2 base col; G-sourced taps marked by ('G', base)."""
    f, l = kind == 'f', kind == 'l'

    def FF(fix):
        return {0: {}, 1: {}, **fix}

    def LL(fix):
        return {**fix, 126: {}, 127: {}}

    def BW(taps, ffix=None, lfix=None):
        cf = None
        if f and ffix is not None:
            cf = FF(ffix)
        if l and lfix is not None:
            cf = LL(lfix)
        return _W(taps, cf)

    P22 = _W({}, {2: {2: 1.0}})
    P125 = _W({}, {125: {125: 1.0}})
    P125n = _W({}, {125: {125: -1.0}})
    P = lambda b: ('P', b)
    G = lambda b: ('G', b)
    R = lambda b: ('R', b)
    RES = {A_: RA, B_: RB, C_: RC, D_: RD}

    def with_res(taps):
        # duplicate every P-plane tap onto the residual tile (same W, same delta)
        out = list(taps)
        for wi, (sk, sb), dlt in taps:
            if sk == 'P':
                out.append((wi, R(RES[sb]), dlt))
        return out

    I1 = _W({0: 1.0})
    NegI = _W({0: -1.0})
    Neg2I = _W({0: -2.0})
    D1 = _W({1: 1.0}, LL({125: {}}) if l else None)   # down-shift; zero bottom edge
    U1 = _W({-1: 1.0}, FF({2: {}}) if f else None)    # up-shift; zero top edge
    NegU1 = _W({-1: -1.0}, FF({2: {}}) if f else None)

    T = {}
    # VD: vdif00 = C[i-1]-C[i]  |  vdif11 = B[i]-B[i+1]   (exact: + residual taps)
    T['VD'] = (
        with_res([(BW({-1: 1, 0: -1}, ffix={2: {2: -1}}), P(C_), 0)] + ([(P22, P(A_), 0)] if f else [])),
        with_res([(BW({0: 1, 1: -1}, lfix={125: {125: 1}}), P(B_), 0)] + ([(P125n, P(D_), 0)] if l else [])),
    )
    # C3: A[i-1]-2A[i]+A[i+1]  |  D[i-1]-2D[i]+D[i+1]
    T['C3'] = (
        with_res([(BW({-1: 1, 0: -2, 1: 1}, ffix={2: {2: -1, 3: 1}}, lfix={125: {124: 1, 125: -2}}), P(A_), 0)]
                 + ([(P125, P(C_), 0)] if l else [])),
        with_res([(BW({-1: 1, 0: -2, 1: 1}, ffix={2: {2: -2, 3: 1}}, lfix={125: {124: 1, 125: -1}}), P(D_), 0)]
                 + ([(P22, P(B_), 0)] if f else [])),
    )
    # HD: hdif00 = B[j-1]-B[j]  |  hdif11 = C[j]-C[j+1]   (horizontal, via col deltas)
    T['HD'] = (
        with_res([(I1, P(B_), -1), (NegI, P(B_), 0)]),
        with_res([(I1, P(C_), 0), (NegI, P(C_), 1)]),
    )
    # C2: A[j-1]-2A[j]+A[j+1]  |  D[j-1]-2D[j]+D[j+1]  (edge cols fixed on psum)
    T['C2'] = (
        with_res([(I1, P(A_), -1), (Neg2I, P(A_), 0), (I1, P(A_), 1)]),
        with_res([(I1, P(D_), -1), (Neg2I, P(D_), 0), (I1, P(D_), 1)]),
    )
    # RV: rawv = 0.5*vsum - 0.25*c3 (the vertical green candidate, final scale)
    T['RV'] = (
        [(BW({-1: .5, 0: .5}, ffix={2: {2: .5}}, lfix={125: {124: .5, 125: .25}}), P(C_), 0),
         (BW({-1: -.25, 0: .5, 1: -.25}, ffix={2: {2: .75, 3: -.25}},
             lfix={125: {124: -.25, 125: .5}}), P(A_), 0)],
        [(BW({0: .5, 1: .5}, ffix={2: {2: .25, 3: .5}}, lfix={125: {125: .5}}), P(B_), 0),
         (BW({-1: -.25, 0: .5, 1: -.25}, ffix={2: {2: .5, 3: -.25}},
             lfix={125: {124: -.25, 125: .75}}), P(D_), 0)],
    )
    # U = t1 - 0.5*t2 for the vertical hv pair (R10 | B01):
    #   R10: (A[i]+A[i+1]) - 0.5*(g00[i]+g00[i+1])   (bottom edge: A only / +C patch)
    #   B01: (D[i-1]+D[i]) - 0.5*(g11[i-1]+g11[i])   (top edge: D only / +B patch)
    P125h = _W({}, {125: {125: -0.5}})
    P22h = _W({}, {2: {2: -0.5}})
    T['U'] = (
        [(BW({0: 1, 1: 1}, lfix={125: {125: 1}}), P(A_), 0),
         (BW({0: -.5, 1: -.5}, lfix={125: {125: -.5}}), G(G00), 0)]
        + ([(P125h, P(C_), 0)] if l else []),
        [(BW({-1: 1, 0: 1}, ffix={2: {2: 1}}), P(D_), 0),
         (BW({-1: -.5, 0: -.5}, ffix={2: {2: -.5}}), G(G11), 0)]
        + ([(P22h, P(B_), 0)] if f else []),
    )
    # S: au = A[i+1] (last: 0)  |  dd = D[i-1] (first: 0)
    T['S'] = (
        [(D1, P(A_), 0)],
        [(U1, P(D_), 0)],
    )
    # DM: |R11| au[j+1]-A[j] = A[i+1,j+1]-A[i,j]  |  |B00| D[i,j]-D[i-1,j-1]
    T['DM'] = (
        [(D1, P(A_), 1), (NegI, P(A_), 0)],
        [(I1, P(D_), 0), (NegU1, P(D_), -1)],
    )
    # DN: R11: A[i+1,j]-A[i,j+1]  |  B00: D[i,j-1]-D[i-1,j]
    T['DN'] = (
        [(D1, P(A_), 0), (NegI, P(A_), 1)],
        [(I1, P(D_), -1), (NegU1, P(D_), 0)],
    )
    # CG2 = sp - 2*gnear;  sp_R11 = g00[i,j]+g00[i+1,j+1], sp_B00 = g11[i-1,j-1]+g11[i,j]
    T['CG2'] = (
        [(I1, G(G00), 0), (D1, G(G00), 1)] + ([(P125, P(C_), 1)] if l else [])
        + [(Neg2I, G(G11), 0)],
        [(U1, G(G11), -1)] + ([(P22, P(B_), -1)] if f else [])
        + [(I1, G(G11), 0), (Neg2I, G(G00), 0)],
    )
    # CG3 = sn - 2*gnear;  sn_R11 = g00[i,j+1]+g00[i+1,j], sn_B00 = g11[i-1,j]+g11[i,j-1]
    T['CG3'] = (
        [(I1, G(G00), 1), (D1, G(G00), 0)] + ([(P125, P(C_), 0)] if l else [])
        + [(Neg2I, G(G11), 0)],
        [(U1, G(G11), 0)] + ([(P22, P(B_), 0)] if f else [])
        + [(I1, G(G11), -1), (Neg2I, G(G00), 0)],
    )
    return T


_TAPS = {k: _tap_table(k) for k in ('f', 'm', 'l')}
NW = len(_mats)
WBANDS = np.concatenate([m.astype(BFNP) for m in _mats], axis=1)  # [128, NW*128]


def pv(t, o0, o1, w=768, step=1):
    """Pair view: [128][2 halves at col offsets o0/o1][w cols at `step`]."""
    b = t[:]
    assert o1 > o0, (o0, o1)
    return bass.AP(tensor=b.tensor, offset=b.offset + o0,
                   ap=[[b.ap[0][0], 128], [o1 - o0, 2], [step, w]])


def build_nc():
    nc = bacc.Bacc("TRN2", target_bir_lowering=False, debug=False, num_devices=NCORES)
    x_in = nc.declare_dram_parameter("x", [4, H, H], BF, isOutput=False)
    xr_in = nc.declare_dram_parameter("xr", [4, H, H], BF, isOutput=False)
    wb_in = nc.declare_dram_parameter("wb", [128, NW * 128], BF, isOutput=False)
    out = nc.declare_dram_parameter("out", [3, 2 * H, 2 * H], BF, isOutput=True)
    out_v = out[:].rearrange("c (r two) w -> c r (two w)", two=2)
    x_r = x_in[:].rearrange("c r w -> r c w")
    xr_r = xr_in[:].rearrange("c r w -> r c w")

    with tile.TileContext(nc) as tc, ExitStack() as ctx:
        wpool = ctx.enter_context(tc.tile_pool(name="wpool", bufs=1))
        pl = ctx.enter_context(tc.tile_pool(name="planes", bufs=2))
        gp = ctx.enter_context(tc.tile_pool(name="greens", bufs=2))
        tp = ctx.enter_context(tc.tile_pool(name="tmps", bufs=16))
        qp_ = ctx.enter_context(tc.tile_pool(name="qtmps", bufs=6))
        mp = ctx.enter_context(tc.tile_pool(name="msks", bufs=2))
        op = ctx.enter_context(tc.tile_pool(name="outs", bufs=2))
        pp = ctx.enter_context(tc.tile_pool(name="psums", bufs=1, space="PSUM"))

        WS = wpool.tile([128, NW * 128], BF, tag="ws", name="ws")
        nc.sync.dma_start(WS[:], wb_in[:])

        pending_out = [None]

        def flush_out():
            if pending_out[0] is None:
                return
            Op, p0p, pnp, r0p = pending_out[0]
            nc.sync.dma_start(out_v[0, r0p:r0p + pnp, :], Op[p0p:p0p + pnp, CR:CR + 3072])
            nc.gpsimd.dma_start(out_v[1, r0p:r0p + pnp, :], Op[p0p:p0p + pnp, CG:CG + 3072])
            nc.sync.dma_start(out_v[2, r0p:r0p + pnp, :], Op[p0p:p0p + pnp, CB:CB + 3072])
            pending_out[0] = None

        for si, r0 in enumerate(STRIPS):
            kind = 'f' if si == 0 else ('l' if si == len(STRIPS) - 1 else 'm')
            taps = _TAPS[kind]
            base = r0 - 2

            psum_ctr = [0]
            P2 = pl.tile([128, 6 * PW], BF, tag="P2", name=f"P2_{si}")
            P2r = pl.tile([128, 4 * PW + 2], BF, tag="P2r", name=f"P2r_{si}")
            G2 = gp.tile([128, 2 * PW], BF, tag="G2", name=f"G2_{si}")
            O = op.tile([128, 3 * 3072], BF, tag="O", name=f"O_{si}")

            def tmp(tag):
                return tp.tile([128, 1536], BF, tag="t", name=f"{tag}_{si}")

            def qf(tag):
                return qp_.tile([128, 1536], F, tag="q", name=f"{tag}_{si}")

            def msk(tag):
                return mp.tile([128, 1536], U8, tag=tag, name=f"{tag}_{si}")

            def psum(tag):
                i = psum_ctr[0] % 4
                psum_ctr[0] += 1
                return pp.tile([128, 768], F, tag=f"u{i}", name=f"{tag}_{si}")

            def emit_half(ps, tl):
                for (c0, c1) in ((0, 512), (512, 768)):
                    n = len(tl)
                    for i, (wi, (sk, sb), dlt) in enumerate(tl):
                        src = {'P': P2, 'G': G2, 'R': P2r}[sk]
                        nc.tensor.matmul(
                            ps[:, c0:c1],
                            WS[:, wi * 128:(wi + 1) * 128],
                            src[:, sb + 1 + dlt + c0:sb + 1 + dlt + c1],
                            start=(i == 0), stop=(i == n - 1))

            # ---- input DMA (planes A..D at P2 cols 770..3849) ----
            # compute-engine partition starts must be 32-aligned: memset a whole
            # 32-lane block first, then let the DMA overwrite the loaded lanes.
            if kind == 'f':
                nc.gpsimd.memset(P2[0:32, :], 0.0)
                nc.gpsimd.memset(P2r[0:32, :], 0.0)
            if kind == 'l':
                nc.gpsimd.memset(P2[96:128, :], 0.0)
                nc.gpsimd.memset(P2r[96:128, :], 0.0)
            clo, chi = max(base, 0), min(base + 128, H)
            Pr = P2[:].rearrange("p (six w) -> p six w", six=6)
            nc.sync.dma_start(Pr[clo - base: chi - base, 1:5, 1:1 + H], x_r[clo:chi, :, :])
            _b = P2r[clo - base: chi - base, :]
            Prr = bass.AP(tensor=_b.tensor, offset=_b.offset + RA + 1,
                          ap=[_b.ap[0], [PW, 4], [1, H]])
            nc.sync.dma_start(Prr, xr_r[clo:chi, :, :])
            flush_out()

            # ---- plane halo cols ----
            # A.r and D.l are ZERO (masked-channel replication) — the two wide
            # ops that instead need the mosaic replication (c2 pair) get tiny
            # single-column fixups below.
            cc = nc.vector.tensor_copy
            cc(P2[:, A_:A_ + 1], P2[:, A_ + 1:A_ + 2])          # A.l = A[0]
            cc(P2[:, B_:B_ + 1], P2[:, A_ + 1:A_ + 2])          # B.l = A[0]
            nc.vector.memset(P2[:, D_:D_ + 1], 0.0)             # D.l = 0
            nc.vector.memset(P2[:, A_ + 769:A_ + 770], 0.0)     # A.r = 0
            cc(P2[:, C_ + 769:C_ + 770], P2[:, D_ + 768:D_ + 769])  # C.r = D[767]
            cc(P2[:, D_ + 769:D_ + 770], P2[:, D_ + 768:D_ + 769])  # D.r = D[767]
            cc(P2r[:, RA:RA + 1], P2r[:, RA + 1:RA + 2])
            cc(P2r[:, RB:RB + 1], P2r[:, RA + 1:RA + 2])
            nc.vector.memset(P2r[:, RD:RD + 1], 0.0)
            nc.vector.memset(P2r[:, RA + 769:RA + 770], 0.0)
            cc(P2r[:, RC + 769:RC + 770], P2r[:, RD + 768:RD + 769])
            cc(P2r[:, RD + 769:RD + 770], P2r[:, RD + 768:RD + 769])
            # green halo cols (read only P2; emitted early so G-dependent PE
            # matmuls wait only on the green interior writes)
            cc(G2[:, G00 + 769:G00 + 770], P2[:, B_ + 768:B_ + 769])  # g00.r = B[767]
            cc(G2[:, G11:G11 + 1], P2[:, C_ + 1:C_ + 2])              # g11.l = C[0]

            # ---- green: PE verticals (consumers emitted in psum-alloc order
            # so the in-order ACT queue can never cycle with PE buffer reuse) ----
            tV = taps['VD']; tC = taps['C3']; tH = taps['HD']; tW = taps['C2']
            psVD0 = psum("vd0"); emit_half(psVD0, tV[0])
            q5 = qf("q5"); nc.scalar.activation(q5[:, 0:768], psVD0[:], AF.Abs)
            psVD1 = psum("vd1"); emit_half(psVD1, tV[1])
            nc.scalar.activation(q5[:, 768:1536], psVD1[:], AF.Abs)
            psC30 = psum("c30"); emit_half(psC30, tC[0])
            q3 = qf("q3"); nc.scalar.activation(q3[:, 0:768], psC30[:], AF.Abs)
            psC31 = psum("c31"); emit_half(psC31, tC[1])
            nc.scalar.activation(q3[:, 768:1536], psC31[:], AF.Abs)

            # ---- green: horizontals ----
            hs = tmp("hs"); nc.vector.tensor_tensor(hs[:], pv(P2, B_, C_ + 1), pv(P2, B_ + 1, C_ + 2), AL.add)
            psHD0 = psum("hd0"); emit_half(psHD0, tH[0])
            q4 = qf("q4"); nc.scalar.activation(q4[:, 0:768], psHD0[:], AF.Abs)
            psHD1 = psum("hd1"); emit_half(psHD1, tH[1])
            nc.scalar.activation(q4[:, 768:1536], psHD1[:], AF.Abs)
            psC20 = psum("c20"); emit_half(psC20, tW[0])
            # c2 wants mosaic replication at the zeroed A.r/D.l halos (main+residual):
            nc.vector.tensor_tensor(psC20[:, 767:768], psC20[:, 767:768], P2[:, B_ + 768:B_ + 769], AL.add)
            nc.vector.tensor_tensor(psC20[:, 767:768], psC20[:, 767:768], P2r[:, RB + 768:RB + 769], AL.add)
            b2 = tmp("b2"); nc.vector.scalar_tensor_tensor(b2[:, 0:768], psC20[:], -0.5, hs[:, 0:768], AL.mult, AL.add)
            q2 = qf("q2"); nc.scalar.activation(q2[:, 0:768], psC20[:], AF.Abs)
            psC21 = psum("c21"); emit_half(psC21, tW[1])
            nc.vector.tensor_tensor(psC21[:, 0:1], psC21[:, 0:1], P2[:, C_ + 1:C_ + 2], AL.add)
            nc.vector.tensor_tensor(psC21[:, 0:1], psC21[:, 0:1], P2r[:, RC + 1:RC + 2], AL.add)
            nc.vector.scalar_tensor_tensor(b2[:, 768:1536], psC21[:], -0.5, hs[:, 768:1536], AL.mult, AL.add)
            nc.scalar.activation(q2[:, 768:1536], psC21[:], AF.Abs)

            clh = qf("clh"); nc.gpsimd.tensor_tensor(clh[:], q4[:], q2[:], AL.add)
            clv = qf("clv"); nc.gpsimd.tensor_tensor(clv[:], q5[:], q3[:], AL.add)
            mk = msk("mk"); nc.vector.tensor_tensor(mk[:], clh[:], clv[:], AL.is_gt)

            tR = taps['RV']
            gint = pv(G2, G00 + 1, G11 + 1)
            nc.vector.tensor_scalar(gint, b2[:], 0.5, None, AL.mult)
            psRV0 = psum("rv0"); emit_half(psRV0, tR[0])
            nc.vector.copy_predicated(G2[:, G00 + 1:G00 + 769], mk[:, 0:768], psRV0[:])
            psRV1 = psum("rv1"); emit_half(psRV1, tR[1])
            nc.vector.copy_predicated(G2[:, G11 + 1:G11 + 769], mk[:, 768:1536], psRV1[:])


            # green outputs: raw (0,1),(1,0) and computed (0,0),(1,1)
            nc.gpsimd.tensor_copy(pv(O, CG + 1, CG + 1536, step=2), pv(P2, B_ + 1, C_ + 1))
            nc.gpsimd.tensor_copy(pv(O, CG, CG + 1537, step=2), gint)

            # ---- chan (R11 | B00) ----
            tS = taps['S']
            psS0 = psum("s0"); emit_half(psS0, tS[0])
            nc.scalar.copy(P2[:, AU + 1:AU + 769], psS0[:])
            nc.vector.memset(P2[:, AU + 769:AU + 770], 0.0)
            psS1 = psum("s1"); emit_half(psS1, tS[1])
            nc.scalar.copy(P2[:, DD + 1:DD + 769], psS1[:])
            nc.vector.memset(P2[:, DD:DD + 1], 0.0)

            rp = tmp("rp"); nc.gpsimd.tensor_tensor(rp[:], pv(P2, A_ + 1, DD), pv(P2, AU + 2, D_ + 1), AL.add)
            rn = tmp("rn"); nc.gpsimd.tensor_tensor(rn[:], pv(P2, A_ + 2, DD + 1), pv(P2, AU + 1, D_), AL.add)

            tDM = taps['DM']; tDN = taps['DN']
            psDM0 = psum("dm0"); emit_half(psDM0, tDM[0])
            qdm = qf("qdm"); nc.scalar.activation(qdm[:, 0:768], psDM0[:], AF.Abs)
            psDM1 = psum("dm1"); emit_half(psDM1, tDM[1])
            nc.scalar.activation(qdm[:, 768:1536], psDM1[:], AF.Abs)
            psDN0 = psum("dn0"); emit_half(psDN0, tDN[0])
            qdn = qf("qdn"); nc.scalar.activation(qdn[:, 0:768], psDN0[:], AF.Abs)
            psDN1 = psum("dn1"); emit_half(psDN1, tDN[1])
            nc.scalar.activation(qdn[:, 768:1536], psDN1[:], AF.Abs)

            # ---- hv fields ----
            tU = taps['U']
            psU0 = psum("u0"); emit_half(psU0, tU[0])
            psU1 = psum("u1"); emit_half(psU1, tU[1])

            t1h = tmp("t1h"); nc.vector.tensor_tensor(t1h[:], pv(P2, A_ + 1, D_), pv(P2, A_ + 2, D_ + 1), AL.add)
            t2h = tmp("t2h"); nc.vector.tensor_tensor(t2h[:], pv(G2, G00 + 1, G11), pv(G2, G00 + 2, G11 + 1), AL.add)
            uh = tmp("uh"); nc.vector.scalar_tensor_tensor(uh[:], t2h[:], -0.5, t1h[:], AL.mult, AL.add)

            # hbc halves are 0.5*B | 0.5*C; the R10/B01 osl ops use them swapped
            hbc = tmp("hbc"); nc.vector.tensor_scalar(hbc[:], pv(P2, B_ + 1, C_ + 1), 0.5, None, AL.mult)

            nc.vector.scalar_tensor_tensor(pv(O, CR + 1, CB + 1536, step=2), uh[:], 0.5, hbc[:], AL.mult, AL.add)
            _d = pv(O, CR + 1536, CB + 1, step=2)
            _d0 = bass.AP(tensor=_d.tensor, offset=_d.offset, ap=[_d.ap[0], [2, 768]])
            _d1 = bass.AP(tensor=_d.tensor, offset=_d.offset + (CB + 1 - (CR + 1536)), ap=[_d.ap[0], [2, 768]])
            nc.vector.scalar_tensor_tensor(_d0, psU0[:], 0.5, hbc[:, 768:1536], AL.mult, AL.add)
            nc.vector.scalar_tensor_tensor(_d1, psU1[:], 0.5, hbc[:, 0:768], AL.mult, AL.add)

            tG2_ = taps['CG2']; tG3_ = taps['CG3']
            psCG20 = psum("g20"); emit_half(psCG20, tG2_[0])
            qp = qf("qp"); nc.scalar.activation(qp[:, 0:768], psCG20[:], AF.Abs)
            cp2 = tmp("cp2"); nc.vector.scalar_tensor_tensor(cp2[:, 0:768], psCG20[:], -0.5, rp[:, 0:768], AL.mult, AL.add)
            psCG21 = psum("g21"); emit_half(psCG21, tG2_[1])
            nc.scalar.activation(qp[:, 768:1536], psCG21[:], AF.Abs)
            nc.vector.scalar_tensor_tensor(cp2[:, 768:1536], psCG21[:], -0.5, rp[:, 768:1536], AL.mult, AL.add)
            psCG30 = psum("g30"); emit_half(psCG30, tG3_[0])
            qn = qf("qn"); nc.scalar.activation(qn[:, 0:768], psCG30[:], AF.Abs)
            cn2 = tmp("cn2"); nc.vector.scalar_tensor_tensor(cn2[:, 0:768], psCG30[:], -0.5, rn[:, 0:768], AL.mult, AL.add)
            psCG31 = psum("g31"); emit_half(psCG31, tG3_[1])
            nc.scalar.activation(qn[:, 768:1536], psCG31[:], AF.Abs)
            nc.vector.scalar_tensor_tensor(cn2[:, 768:1536], psCG31[:], -0.5, rn[:, 768:1536], AL.mult, AL.add)
            clp = qf("clp"); nc.gpsimd.tensor_tensor(clp[:], qdm[:], qp[:], AL.add)
            cln = qf("cln"); nc.gpsimd.tensor_tensor(cln[:], qdn[:], qn[:], AL.add)
            mr = msk("mr"); nc.vector.tensor_tensor(mr[:], clp[:], cln[:], AL.is_gt)
            cnh = tmp("cnh"); nc.vector.tensor_scalar(cnh[:], cn2[:], 0.5, None, AL.mult)
            odst = pv(O, CR + 1537, CB, step=2)
            nc.scalar.mul(odst, cp2[:], 0.5)
            nc.vector.copy_predicated(odst, mr[:], cnh[:])

            # ---- raw red/blue phases ----
            nc.gpsimd.tensor_copy(pv(O, CR, CB + 1537, step=2), pv(P2, A_ + 1, D_ + 1))

            # ---- output DMA (3 colors on 3 queues), deferred so the NEXT
            # strip's input DMAs precede them on the SP queue ----
            if kind == 'l':
                p0, pn, row0 = 102, 24, 744
            else:
                p0, pn, row0 = 2, 124, r0
            pending_out[0] = (O, p0, pn, row0)
        flush_out()

    nc.compile()
    return nc


_NC_CACHE = None


def kernel(x: np.ndarray) -> np.ndarray:
    global _NC_CACHE
    if _NC_CACHE is None:
        _NC_CACHE = build_nc()
    xb = np.ascontiguousarray(x.astype(BFNP))
    xr = np.ascontiguousarray((x.astype(np.float32) - xb.astype(np.float32)).astype(BFNP))
    wb = np.ascontiguousarray(WBANDS)
    in_maps = [{"x": xb[i], "xr": xr[i], "wb": wb} for i in range(NCORES)]
    res = run_bass_kernel_spmd(_NC_CACHE, in_maps, list(range(NCORES)))
    return np.stack([res.results[i]["out"].astype(np.float32) for i in range(NCORES)], axis=0)


# revision 11
# speedup vs baseline: 1.1954x; 1.0046x over previous
"""Hamilton-Adams demosaic for Trainium2 — v2 (bf16, PE banded verticals, paired ops).

Input:  x [8, 4, 768, 768] f32  (quarter-res planes A=R, B=Gr, C=Gb, D=B)
Output: [8, 3, 1536, 1536] f32

All device compute in bf16 (host casts in/out).  Every gradient feeding a
directional-selection compare (vdif, c3, hdif, c2, dm, dn) runs on the Tensor
engine as a banded-stationary matmul accumulating BOTH the bf16 input and a
bf16 residual channel (xr = x - bf16(x)) in fp32 PSUM, so selection decisions
are effectively fp32-exact; only output values carry bf16 noise (l2 ~8e-3,
budget 2e-2).  Cross-partition (vertical) stencils and shifts are all PE
matmuls (W [128,128] per band, edge strips get modified bands + patch taps);
horizontal value ops run on DVE; |x| on Act; compares/copies on Pool.  The two
symmetric phases of each computation are fused into single wide ops via 3-dim
access patterns with a per-half column offset; PSUM results use 2-bank
single-half tiles on a 4-tag rotation for pipeline depth.

Tile layouts (halo col each side of every 770-wide plane region):
  P2  [128, 6*770] bf16: [au | A | B | C | D | dd]  (au=A row+1, dd=D row-1)
  P2r [128, 4*770+2] bf16: residual planes [A | B | C | D]
  G2  [128, 2*770] bf16: [g00 | g11]
  O   [128, 3*3072] bf16: interleaved full-res output rows (R | G | B)
"""
import sys
sys.path.insert(0, '/opt/trn_rl_repo')

from contextlib import ExitStack

import numpy as np
import ml_dtypes

import concourse.bass as bass
import concourse.bacc as bacc
import concourse.tile as tile
from concourse import mybir
from concourse.bass_utils import run_bass_kernel_spmd

F = mybir.dt.float32
BF = mybir.dt.bfloat16
U8 = mybir.dt.uint8
AL = mybir.AluOpType
AF = mybir.ActivationFunctionType
BFNP = ml_dtypes.bfloat16

H = 768
PW = 770
NCORES = 8
STRIPS = [0, 124, 248, 372, 496, 620, 644]

# P2 region base cols
AU, A_, B_, C_, D_, DD = 0, 770, 1540, 2310, 3080, 3850
# P2r (bf16 residual planes) region base cols — offset by 1 so taps with
# column delta -1/+1 on the first/last region stay inside the tile
RA, RB, RC, RD = 1, 771, 1541, 2311
# G2 region base cols
G00, G11 = 0, 770
# O color base cols
CR, CG, CB = 0, 3072, 6144

# ---------------- host-side band matrices ----------------
_mats: list[np.ndarray] = []
_mat_idx: dict = {}


def _W(taps=None, colfix=None):
    taps = taps or {}
    colfix = colfix or {}
    key = (tuple(sorted(taps.items())),
           tuple(sorted((m, tuple(sorted(f.items()))) for m, f in colfix.items())))
    if key in _mat_idx:
        return _mat_idx[key]
    M = np.zeros((128, 128), np.float32)
    for m in range(128):
        if m in colfix:
            for k, w in colfix[m].items():
                M[k, m] = w
        else:
            for d, w in taps.items():
                k = m + d
                if 0 <= k < 128:
                    M[k, m] = w
    idx = len(_mats)
    _mats.append(M)
    _mat_idx[key] = idx
    return idx


def _tap_table(kind):
    """kind in 'f' (first strip), 'm' (mid), 'l' (last).
    Returns result -> (half0 taps, half1 taps); tap = (Widx, src_region).
    src_region is a P2 or G2 base col; G-sourced taps marked by ('G', base)."""
    f, l = kind == 'f', kind == 'l'

    def FF(fix):
        return {0: {}, 1: {}, **fix}

    def LL(fix):
        return {**fix, 126: {}, 127: {}}

    def BW(taps, ffix=None, lfix=None):
        cf = None
        if f and ffix is not None:
            cf = FF(ffix)
        if l and lfix is not None:
            cf = LL(lfix)
        return _W(taps, cf)

    P22 = _W({}, {2: {2: 1.0}})
    P125 = _W({}, {125: {125: 1.0}})
    P125n = _W({}, {125: {125: -1.0}})
    P = lambda b: ('P', b)
    G = lambda b: ('G', b)
    R = lambda b: ('R', b)
    RES = {A_: RA, B_: RB, C_: RC, D_: RD}

    def with_res(taps):
        # duplicate every P-plane tap onto the residual tile (same W, same delta)
        out = list(taps)
        for wi, (sk, sb), dlt in taps:
            if sk == 'P':
                out.append((wi, R(RES[sb]), dlt))
        return out

    I1 = _W({0: 1.0})
    NegI = _W({0: -1.0})
    Neg2I = _W({0: -2.0})
    D1 = _W({1: 1.0}, LL({125: {}}) if l else None)   # down-shift; zero bottom edge
    U1 = _W({-1: 1.0}, FF({2: {}}) if f else None)    # up-shift; zero top edge
    NegU1 = _W({-1: -1.0}, FF({2: {}}) if f else None)

    T = {}
    # VD: vdif00 = C[i-1]-C[i]  |  vdif11 = B[i]-B[i+1]   (exact: + residual taps)
    T['VD'] = (
        with_res([(BW({-1: 1, 0: -1}, ffix={2: {2: -1}}), P(C_), 0)] + ([(P22, P(A_), 0)] if f else [])),
        with_res([(BW({0: 1, 1: -1}, lfix={125: {125: 1}}), P(B_), 0)] + ([(P125n, P(D_), 0)] if l else [])),
    )
    # C3: A[i-1]-2A[i]+A[i+1]  |  D[i-1]-2D[i]+D[i+1]
    T['C3'] = (
        with_res([(BW({-1: 1, 0: -2, 1: 1}, ffix={2: {2: -1, 3: 1}}, lfix={125: {124: 1, 125: -2}}), P(A_), 0)]
                 + ([(P125, P(C_), 0)] if l else [])),
        with_res([(BW({-1: 1, 0: -2, 1: 1}, ffix={2: {2: -2, 3: 1}}, lfix={125: {124: 1, 125: -1}}), P(D_), 0)]
                 + ([(P22, P(B_), 0)] if f else [])),
    )
    # HD: hdif00 = B[j-1]-B[j]  |  hdif11 = C[j]-C[j+1]   (horizontal, via col deltas)
    T['HD'] = (
        with_res([(I1, P(B_), -1), (NegI, P(B_), 0)]),
        with_res([(I1, P(C_), 0), (NegI, P(C_), 1)]),
    )
    # C2: A[j-1]-2A[j]+A[j+1]  |  D[j-1]-2D[j]+D[j+1]  (edge cols fixed on psum)
    T['C2'] = (
        with_res([(I1, P(A_), -1), (Neg2I, P(A_), 0), (I1, P(A_), 1)]),
        with_res([(I1, P(D_), -1), (Neg2I, P(D_), 0), (I1, P(D_), 1)]),
    )
    # RV: rawv = 0.5*vsum - 0.25*c3 (the vertical green candidate, final scale)
    T['RV'] = (
        [(BW({-1: .5, 0: .5}, ffix={2: {2: .5}}, lfix={125: {124: .5, 125: .25}}), P(C_), 0),
         (BW({-1: -.25, 0: .5, 1: -.25}, ffix={2: {2: .75, 3: -.25}},
             lfix={125: {124: -.25, 125: .5}}), P(A_), 0)],
        [(BW({0: .5, 1: .5}, ffix={2: {2: .25, 3: .5}}, lfix={125: {125: .5}}), P(B_), 0),
         (BW({-1: -.25, 0: .5, 1: -.25}, ffix={2: {2: .5, 3: -.25}},
             lfix={125: {124: -.25, 125: .75}}), P(D_), 0)],
    )
    # U = t1 - 0.5*t2 for the vertical hv pair (R10 | B01):
    #   R10: (A[i]+A[i+1]) - 0.5*(g00[i]+g00[i+1])   (bottom edge: A only / +C patch)
    #   B01: (D[i-1]+D[i]) - 0.5*(g11[i-1]+g11[i])   (top edge: D only / +B patch)
    P125h = _W({}, {125: {125: -0.5}})
    P22h = _W({}, {2: {2: -0.5}})
    T['U'] = (
        [(BW({0: 1, 1: 1}, lfix={125: {125: 1}}), P(A_), 0),
         (BW({0: -.5, 1: -.5}, lfix={125: {125: -.5}}), G(G00), 0)]
        + ([(P125h, P(C_), 0)] if l else []),
        [(BW({-1: 1, 0: 1}, ffix={2: {2: 1}}), P(D_), 0),
         (BW({-1: -.5, 0: -.5}, ffix={2: {2: -.5}}), G(G11), 0)]
        + ([(P22h, P(B_), 0)] if f else []),
    )
    # S: au = A[i+1] (last: 0)  |  dd = D[i-1] (first: 0)
    T['S'] = (
        [(D1, P(A_), 0)],
        [(U1, P(D_), 0)],
    )
    # DM: |R11| au[j+1]-A[j] = A[i+1,j+1]-A[i,j]  |  |B00| D[i,j]-D[i-1,j-1]
    T['DM'] = (
        [(D1, P(A_), 1), (NegI, P(A_), 0)],
        [(I1, P(D_), 0), (NegU1, P(D_), -1)],
    )
    # DN: R11: A[i+1,j]-A[i,j+1]  |  B00: D[i,j-1]-D[i-1,j]
    T['DN'] = (
        [(D1, P(A_), 0), (NegI, P(A_), 1)],
        [(I1, P(D_), -1), (NegU1, P(D_), 0)],
    )
    # CG2 = sp - 2*gnear;  sp_R11 = g00[i,j]+g00[i+1,j+1], sp_B00 = g11[i-1,j-1]+g11[i,j]
    T['CG2'] = (
        [(I1, G(G00), 0), (D1, G(G00), 1)] + ([(P125, P(C_), 1)] if l else [])
        + [(Neg2I, G(G11), 0)],
        [(U1, G(G11), -1)] + ([(P22, P(B_), -1)] if f else [])
        + [(I1, G(G11), 0), (Neg2I, G(G00), 0)],
    )
    # CG3 = sn - 2*gnear;  sn_R11 = g00[i,j+1]+g00[i+1,j], sn_B00 = g11[i-1,j]+g11[i,j-1]
    T['CG3'] = (
        [(I1, G(G00), 1), (D1, G(G00), 0)] + ([(P125, P(C_), 0)] if l else [])
        + [(Neg2I, G(G11), 0)],
        [(U1, G(G11), 0)] + ([(P22, P(B_), 0)] if f else [])
        + [(I1, G(G11), -1), (Neg2I, G(G00), 0)],
    )
    return T


_TAPS = {k: _tap_table(k) for k in ('f', 'm', 'l')}
NW = len(_mats)
WBANDS = np.concatenate([m.astype(BFNP) for m in _mats], axis=1)  # [128, NW*128]


def pv(t, o0, o1, w=768, step=1):
    """Pair view: [128][2 halves at col offsets o0/o1][w cols at `step`]."""
    b = t[:]
    assert o1 > o0, (o0, o1)
    return bass.AP(tensor=b.tensor, offset=b.offset + o0,
                   ap=[[b.ap[0][0], 128], [o1 - o0, 2], [step, w]])


def build_nc():
    nc = bacc.Bacc("TRN2", target_bir_lowering=False, debug=False, num_devices=NCORES)
    x_in = nc.declare_dram_parameter("x", [4, H, H], BF, isOutput=False)
    xr_in = nc.declare_dram_parameter("xr", [4, H, H], BF, isOutput=False)
    wb_in = nc.declare_dram_parameter("wb", [128, NW * 128], BF, isOutput=False)
    out = nc.declare_dram_parameter("out", [3, 2 * H, 2 * H], BF, isOutput=True)
    out_v = out[:].rearrange("c (r two) w -> c r (two w)", two=2)
    x_r = x_in[:].rearrange("c r w -> r c w")
    xr_r = xr_in[:].rearrange("c r w -> r c w")

    with tile.TileContext(nc) as tc, ExitStack() as ctx:
        wpool = ctx.enter_context(tc.tile_pool(name="wpool", bufs=1))
        pl = ctx.enter_context(tc.tile_pool(name="planes", bufs=2))
        gp = ctx.enter_context(tc.tile_pool(name="greens", bufs=2))
        tp = ctx.enter_context(tc.tile_pool(name="tmps", bufs=16))
        qp_ = ctx.enter_context(tc.tile_pool(name="qtmps", bufs=6))
        mp = ctx.enter_context(tc.tile_pool(name="msks", bufs=2))
        op = ctx.enter_context(tc.tile_pool(name="outs", bufs=2))
        pp = ctx.enter_context(tc.tile_pool(name="psums", bufs=1, space="PSUM"))

        WS = wpool.tile([128, NW * 128], BF, tag="ws", name="ws")
        nc.scalar.dma_start(WS[:], wb_in[:])

        pending_out = [None]

        def flush_out():
            if pending_out[0] is None:
                return
            Op, p0p, pnp, r0p = pending_out[0]
            nc.sync.dma_start(out_v[0, r0p:r0p + pnp, :], Op[p0p:p0p + pnp, CR:CR + 3072])
            nc.gpsimd.dma_start(out_v[1, r0p:r0p + pnp, :], Op[p0p:p0p + pnp, CG:CG + 3072])
            nc.sync.dma_start(out_v[2, r0p:r0p + pnp, :], Op[p0p:p0p + pnp, CB:CB + 3072])
            pending_out[0] = None

        for si, r0 in enumerate(STRIPS):
            kind = 'f' if si == 0 else ('l' if si == len(STRIPS) - 1 else 'm')
            taps = _TAPS[kind]
            base = r0 - 2

            psum_ctr = [0]
            P2 = pl.tile([128, 6 * PW], BF, tag="P2", name=f"P2_{si}")
            P2r = pl.tile([128, 4 * PW + 2], BF, tag="P2r", name=f"P2r_{si}")
            G2 = gp.tile([128, 2 * PW], BF, tag="G2", name=f"G2_{si}")
            O = op.tile([128, 3 * 3072], BF, tag="O", name=f"O_{si}")

            def tmp(tag):
                return tp.tile([128, 1536], BF, tag="t", name=f"{tag}_{si}")

            def qf(tag):
                return qp_.tile([128, 1536], F, tag="q", name=f"{tag}_{si}")

            def msk(tag):
                return mp.tile([128, 1536], U8, tag=tag, name=f"{tag}_{si}")

            def psum(tag):
                i = psum_ctr[0] % 4
                psum_ctr[0] += 1
                return pp.tile([128, 768], F, tag=f"u{i}", name=f"{tag}_{si}")

            def emit_half(ps, tl):
                for (c0, c1) in ((0, 512), (512, 768)):
                    n = len(tl)
                    for i, (wi, (sk, sb), dlt) in enumerate(tl):
                        src = {'P': P2, 'G': G2, 'R': P2r}[sk]
                        nc.tensor.matmul(
                            ps[:, c0:c1],
                            WS[:, wi * 128:(wi + 1) * 128],
                            src[:, sb + 1 + dlt + c0:sb + 1 + dlt + c1],
                            start=(i == 0), stop=(i == n - 1))

            # ---- input DMA (planes A..D at P2 cols 770..3849) ----
            # compute-engine partition starts must be 32-aligned: memset a whole
            # 32-lane block first, then let the DMA overwrite the loaded lanes.
            if kind == 'f':
                nc.gpsimd.memset(P2[0:32, :], 0.0)
                nc.gpsimd.memset(P2r[0:32, :], 0.0)
            if kind == 'l':
                nc.gpsimd.memset(P2[96:128, :], 0.0)
                nc.gpsimd.memset(P2r[96:128, :], 0.0)
            clo, chi = max(base, 0), min(base + 128, H)
            Pr = P2[:].rearrange("p (six w) -> p six w", six=6)
            nc.sync.dma_start(Pr[clo - base: chi - base, 1:5, 1:1 + H], x_r[clo:chi, :, :])
            _b = P2r[clo - base: chi - base, :]
            Prr = bass.AP(tensor=_b.tensor, offset=_b.offset + RA + 1,
                          ap=[_b.ap[0], [PW, 4], [1, H]])
            nc.sync.dma_start(Prr, xr_r[clo:chi, :, :])
            flush_out()

            # ---- plane halo cols ----
            # A.r and D.l are ZERO (masked-channel replication) — the two wide
            # ops that instead need the mosaic replication (c2 pair) get tiny
            # single-column fixups below.
            cc = nc.vector.tensor_copy
            cc(P2[:, A_:A_ + 1], P2[:, A_ + 1:A_ + 2])          # A.l = A[0]
            cc(P2[:, B_:B_ + 1], P2[:, A_ + 1:A_ + 2])          # B.l = A[0]
            nc.vector.memset(P2[:, D_:D_ + 1], 0.0)             # D.l = 0
            nc.vector.memset(P2[:, A_ + 769:A_ + 770], 0.0)     # A.r = 0
            cc(P2[:, C_ + 769:C_ + 770], P2[:, D_ + 768:D_ + 769])  # C.r = D[767]
            cc(P2[:, D_ + 769:D_ + 770], P2[:, D_ + 768:D_ + 769])  # D.r = D[767]
            cc(P2r[:, RA:RA + 1], P2r[:, RA + 1:RA + 2])
            cc(P2r[:, RB:RB + 1], P2r[:, RA + 1:RA + 2])
            nc.vector.memset(P2r[:, RD:RD + 1], 0.0)
            nc.vector.memset(P2r[:, RA + 769:RA + 770], 0.0)
            cc(P2r[:, RC + 769:RC + 770], P2r[:, RD + 768:RD + 769])
            cc(P2r[:, RD + 769:RD + 770], P2r[:, RD + 768:RD + 769])
            # green halo cols (read only P2; emitted early so G-dependent PE
            # matmuls wait only on the green interior writes)
            cc(G2[:, G00 + 769:G00 + 770], P2[:, B_ + 768:B_ + 769])  # g00.r = B[767]
            cc(G2[:, G11:G11 + 1], P2[:, C_ + 1:C_ + 2])              # g11.l = C[0]

            # ---- green: PE verticals (consumers emitted in psum-alloc order
            # so the in-order ACT queue can never cycle with PE buffer reuse) ----
            tV = taps['VD']; tC = taps['C3']; tH = taps['HD']; tW = taps['C2']
            psVD0 = psum("vd0"); emit_half(psVD0, tV[0])
            q5 = qf("q5"); nc.scalar.activation(q5[:, 0:768], psVD0[:], AF.Abs)
            psVD1 = psum("vd1"); emit_half(psVD1, tV[1])
            nc.scalar.activation(q5[:, 768:1536], psVD1[:], AF.Abs)
            psC30 = psum("c30"); emit_half(psC30, tC[0])
            q3 = qf("q3"); nc.scalar.activation(q3[:, 0:768], psC30[:], AF.Abs)
            psC31 = psum("c31"); emit_half(psC31, tC[1])
            nc.scalar.activation(q3[:, 768:1536], psC31[:], AF.Abs)

            # ---- green: horizontals ----
            hs = tmp("hs"); nc.vector.tensor_tensor(hs[:], pv(P2, B_, C_ + 1), pv(P2, B_ + 1, C_ + 2), AL.add)
            psHD0 = psum("hd0"); emit_half(psHD0, tH[0])
            q4 = qf("q4"); nc.scalar.activation(q4[:, 0:768], psHD0[:], AF.Abs)
            psHD1 = psum("hd1"); emit_half(psHD1, tH[1])
            nc.scalar.activation(q4[:, 768:1536], psHD1[:], AF.Abs)
            psC20 = psum("c20"); emit_half(psC20, tW[0])
            # c2 wants mosaic replication at the zeroed A.r/D.l halos (main+residual):
            nc.vector.tensor_tensor(psC20[:, 767:768], psC20[:, 767:768], P2[:, B_ + 768:B_ + 769], AL.add)
            nc.vector.tensor_tensor(psC20[:, 767:768], psC20[:, 767:768], P2r[:, RB + 768:RB + 769], AL.add)
            b2 = tmp("b2"); nc.vector.scalar_tensor_tensor(b2[:, 0:768], psC20[:], -0.5, hs[:, 0:768], AL.mult, AL.add)
            q2 = qf("q2"); nc.scalar.activation(q2[:, 0:768], psC20[:], AF.Abs)
            psC21 = psum("c21"); emit_half(psC21, tW[1])
            nc.vector.tensor_tensor(psC21[:, 0:1], psC21[:, 0:1], P2[:, C_ + 1:C_ + 2], AL.add)
            nc.vector.tensor_tensor(psC21[:, 0:1], psC21[:, 0:1], P2r[:, RC + 1:RC + 2], AL.add)
            nc.vector.scalar_tensor_tensor(b2[:, 768:1536], psC21[:], -0.5, hs[:, 768:1536], AL.mult, AL.add)
            nc.scalar.activation(q2[:, 768:1536], psC21[:], AF.Abs)

            clh = qf("clh"); nc.gpsimd.tensor_tensor(clh[:], q4[:], q2[:], AL.add)
            clv = qf("clv"); nc.gpsimd.tensor_tensor(clv[:], q5[:], q3[:], AL.add)
            mk = msk("mk"); nc.vector.tensor_tensor(mk[:], clh[:], clv[:], AL.is_gt)

            tR = taps['RV']
            gint = pv(G2, G00 + 1, G11 + 1)
            nc.vector.tensor_scalar(gint, b2[:], 0.5, None, AL.mult)
            psRV0 = psum("rv0"); emit_half(psRV0, tR[0])
            nc.vector.copy_predicated(G2[:, G00 + 1:G00 + 769], mk[:, 0:768], psRV0[:])
            psRV1 = psum("rv1"); emit_half(psRV1, tR[1])
            nc.vector.copy_predicated(G2[:, G11 + 1:G11 + 769], mk[:, 768:1536], psRV1[:])


            # green outputs: raw (0,1),(1,0) and computed (0,0),(1,1)
            nc.gpsimd.tensor_copy(pv(O, CG + 1, CG + 1536, step=2), pv(P2, B_ + 1, C_ + 1))
            nc.gpsimd.tensor_copy(pv(O, CG, CG + 1537, step=2), gint)

            # ---- chan (R11 | B00) ----
            tS = taps['S']
            psS0 = psum("s0"); emit_half(psS0, tS[0])
            nc.scalar.copy(P2[:, AU + 1:AU + 769], psS0[:])
            nc.vector.memset(P2[:, AU + 769:AU + 770], 0.0)
            psS1 = psum("s1"); emit_half(psS1, tS[1])
            nc.scalar.copy(P2[:, DD + 1:DD + 769], psS1[:])
            nc.vector.memset(P2[:, DD:DD + 1], 0.0)

            rp = tmp("rp"); nc.gpsimd.tensor_tensor(rp[:], pv(P2, A_ + 1, DD), pv(P2, AU + 2, D_ + 1), AL.add)
            rn = tmp("rn"); nc.gpsimd.tensor_tensor(rn[:], pv(P2, A_ + 2, DD + 1), pv(P2, AU + 1, D_), AL.add)

            tDM = taps['DM']; tDN = taps['DN']
            psDM0 = psum("dm0"); emit_half(psDM0, tDM[0])
            qdm = qf("qdm"); nc.scalar.activation(qdm[:, 0:768], psDM0[:], AF.Abs)
            psDM1 = psum("dm1"); emit_half(psDM1, tDM[1])
            nc.scalar.activation(qdm[:, 768:1536], psDM1[:], AF.Abs)
            psDN0 = psum("dn0"); emit_half(psDN0, tDN[0])
            qdn = qf("qdn"); nc.scalar.activation(qdn[:, 0:768], psDN0[:], AF.Abs)
            psDN1 = psum("dn1"); emit_half(psDN1, tDN[1])
            nc.scalar.activation(qdn[:, 768:1536], psDN1[:], AF.Abs)

            # ---- hv fields ----
            tU = taps['U']
            psU0 = psum("u0"); emit_half(psU0, tU[0])
            psU1 = psum("u1"); emit_half(psU1, tU[1])

            t1h = tmp("t1h"); nc.vector.tensor_tensor(t1h[:], pv(P2, A_ + 1, D_), pv(P2, A_ + 2, D_ + 1), AL.add)
            t2h = tmp("t2h"); nc.vector.tensor_tensor(t2h[:], pv(G2, G00 + 1, G11), pv(G2, G00 + 2, G11 + 1), AL.add)
            uh = tmp("uh"); nc.vector.scalar_tensor_tensor(uh[:], t2h[:], -0.5, t1h[:], AL.mult, AL.add)

            # hbc halves are 0.5*B | 0.5*C; the R10/B01 osl ops use them swapped
            hbc = tmp("hbc"); nc.vector.tensor_scalar(hbc[:], pv(P2, B_ + 1, C_ + 1), 0.5, None, AL.mult)

            nc.vector.scalar_tensor_tensor(pv(O, CR + 1, CB + 1536, step=2), uh[:], 0.5, hbc[:], AL.mult, AL.add)
            _d = pv(O, CR + 1536, CB + 1, step=2)
            _d0 = bass.AP(tensor=_d.tensor, offset=_d.offset, ap=[_d.ap[0], [2, 768]])
            _d1 = bass.AP(tensor=_d.tensor, offset=_d.offset + (CB + 1 - (CR + 1536)), ap=[_d.ap[0], [2, 768]])
            nc.vector.scalar_tensor_tensor(_d0, psU0[:], 0.5, hbc[:, 768:1536], AL.mult, AL.add)
            nc.vector.scalar_tensor_tensor(_d1, psU1[:], 0.5, hbc[:, 0:768], AL.mult, AL.add)

            tG2_ = taps['CG2']; tG3_ = taps['CG3']
            psCG20 = psum("g20"); emit_half(psCG20, tG2_[0])
            qp = qf("qp"); nc.scalar.activation(qp[:, 0:768], psCG20[:], AF.Abs)
            cp2 = tmp("cp2"); nc.vector.scalar_tensor_tensor(cp2[:, 0:768], psCG20[:], -0.5, rp[:, 0:768], AL.mult, AL.add)
            psCG21 = psum("g21"); emit_half(psCG21, tG2_[1])
            nc.scalar.activation(qp[:, 768:1536], psCG21[:], AF.Abs)
            nc.vector.scalar_tensor_tensor(cp2[:, 768:1536], psCG21[:], -0.5, rp[:, 768:1536], AL.mult, AL.add)
            psCG30 = psum("g30"); emit_half(psCG30, tG3_[0])
            qn = qf("qn"); nc.scalar.activation(qn[:, 0:768], psCG30[:], AF.Abs)
            cn2 = tmp("cn2"); nc.vector.scalar_tensor_tensor(cn2[:, 0:768], psCG30[:], -0.5, rn[:, 0:768], AL.mult, AL.add)
            psCG31 = psum("g31"); emit_half(psCG31, tG3_[1])
            nc.scalar.activation(qn[:, 768:1536], psCG31[:], AF.Abs)
            nc.vector.scalar_tensor_tensor(cn2[:, 768:1536], psCG31[:], -0.5, rn[:, 768:1536], AL.mult, AL.add)
            clp = qf("clp"); nc.gpsimd.tensor_tensor(clp[:], qdm[:], qp[:], AL.add)
            cln = qf("cln"); nc.gpsimd.tensor_tensor(cln[:], qdn[:], qn[:], AL.add)
            mr = msk("mr"); nc.vector.tensor_tensor(mr[:], clp[:], cln[:], AL.is_gt)
            cnh = tmp("cnh"); nc.vector.tensor_scalar(cnh[:], cn2[:], 0.5, None, AL.mult)
            odst = pv(O, CR + 1537, CB, step=2)
            nc.scalar.mul(odst, cp2[:], 0.5)
            nc.vector.copy_predicated(odst, mr[:], cnh[:])

            # ---- raw red/blue phases ----
            nc.gpsimd.tensor_copy(pv(O, CR, CB + 1537, step=2), pv(P2, A_ + 1, D_ + 1))

            # ---- output DMA (3 colors on 3 queues), deferred so the NEXT
            # strip's input DMAs precede them on the SP queue ----
            if kind == 'l':
                p0, pn, row0 = 102, 24, 744
            else:
                p0, pn, row0 = 2, 124, r0
            pending_out[0] = (O, p0, pn, row0)
        flush_out()

    nc.compile()
    return nc


_NC_CACHE = None


def kernel(x: np.ndarray) -> np.ndarray:
    global _NC_CACHE
    if _NC_CACHE is None:
        _NC_CACHE = build_nc()
    xb = np.ascontiguousarray(x.astype(BFNP))
    xr = np.ascontiguousarray((x.astype(np.float32) - xb.astype(np.float32)).astype(BFNP))
    wb = np.ascontiguousarray(WBANDS)
    in_maps = [{"x": xb[i], "xr": xr[i], "wb": wb} for i in range(NCORES)]
    res = run_bass_kernel_spmd(_NC_CACHE, in_maps, list(range(NCORES)))
    return np.stack([res.results[i]["out"].astype(np.float32) for i in range(NCORES)], axis=0)
